# revision 28
# baseline (speedup 1.0000x reference)
"""Trainium2 Bass kernel for nn_Attention_12463995093474 (sparse_attention).

Math (reference):
  q/k/v = content linears; 2 absolute heads, 2 relative heads (DK=32).
  abs:  scores = (Xq_a + abs_kernel@abs_q_w) @ (Xk_a + abs_kernel@abs_k_w)^T
  rel:  scores = Xq_r @ Xk_r^T + (Xq_r + rel_bias) . (rel_kernel@rel_k_w + rel_k_b)
  softmax(mask) @ v -> out linear.

Key optimization: the dominant term
    sum_o (Xq_r+rel_bias)[i,o] * (sum_d rel_kernel[i,j,d] rel_k_w[d,o] + rel_k_b[o])
is reassociated to
    sum_d rel_kernel[i,j,d] * u[i,d] + c[i],
      u = rel_k_w @ (Xq_r+rel_bias)^T,  c = (Xq_r+rel_bias) @ rel_k_b
which turns a 21-GFLOP tensor contraction into a single streaming pass over
rel_kernel (655 MB) in bf16 on the Vector engine: elementwise multiply by u
(free-dim-broadcast AP) + pairwise tree reduction over d.

DVE op cost is free-dim-size bound, so every streaming chunk must use all
128 partitions. Per rel head hr, the 2x200 (batch, i) rows are packed as:
  blocks A/B/C: partition = (b in 2) x (i-halfblock of 64) -> 128 rows
  block D (i in [192,200)): partition = (jq in 8) x (b in 2) x (i in 8),
    with j split in 8 chunks of 25; uses a jq-replicated u operand, and its
    s2 result is unpacked back to row-major with 8 identity-slice matmuls.
All PE writes land on 0/64 partition bases (hardware requires 32-aligned).

Softmax skips the max-subtraction (logits are O(1); masked entries are -1e9
and exp underflows to exactly 0); p normalized on the Scalar engine in bf16.

Sharding: data-parallel over batch, B=16 -> 2 batches per core on 8 cores.
"""

import numpy as np
from contextlib import ExitStack

import concourse.bass as bass
import concourse.bacc as bacc
import concourse.tile as tile
from concourse import mybir
from concourse.masks import make_identity
from concourse.bass_utils import run_bass_kernel_spmd

B, T, D = 16, 200, 128
H_ABS, H_REL, H, DK = 2, 2, 4, 32
N_CORES = 8
BPC = B // N_CORES  # batches per core
SCALE = 1.0 / float(DK) ** 0.5
JC = 50  # j-chunk for full streaming blocks (T % JC == 0)
TT = BPC * T  # tokens per core (400)

F32 = mybir.dt.float32
BF16 = mybir.dt.bfloat16
I32 = mybir.dt.int32
AX = mybir.AxisListType
OP = mybir.AluOpType
AF = mybir.ActivationFunctionType

# abs-head i-blocks per batch: (start, len)
IBLOCKS = [(0, 128), (128, T - 128)]
# full stream blocks per hr: i-halfblocks of 64, two batches stacked
HB = 64
FULL_I0 = [0, 64, 128]
DN = T - 192  # 8 leftover i-rows -> packed block D
DJ = T // 8   # 25: j-chunk for block D


def build_kernel(ctx: ExitStack, tc: tile.TileContext, io: dict):
    nc = tc.nc

    query = io["query"].flatten_outer_dims()  # [400, 128]
    key = io["key"].flatten_outer_dims()
    value = io["value"].flatten_outer_dims()
    mask = io["mask"]          # [2, 1, 200, 200] i32
    relk = io["rel_kernel"]    # [2, 2, 200, 200, 128] bf16
    absk = io["abs_kernel"]    # [2, 2, 200, 128]
    out = io["out"]            # [2, 200, 128]

    consts = ctx.enter_context(tc.tile_pool(name="consts", bufs=1))
    prep = ctx.enter_context(tc.tile_pool(name="prep", bufs=2))
    keep = ctx.enter_context(tc.tile_pool(name="keep", bufs=1))

    def dma_sync(out_ap, in_ap):
        return nc.sync.dma_start(out_ap, in_ap)

    def dma_scalar(out_ap, in_ap):
        return nc.scalar.dma_start(out_ap, in_ap)

    ident = consts.tile([128, 128], F32, tag="ident")
    make_identity(nc, ident)
    identb = consts.tile([128, 128], BF16, tag="identb")
    nc.gpsimd.tensor_copy(identb, ident)

    def load_const(name, ap, shape, eng=dma_scalar):
        t = consts.tile(shape, F32, tag=name)
        eng(t, ap)
        return t

    with tc.tile_pool(name="psum_prep", bufs=2, space="PSUM") as psp, \
         tc.tile_pool(name="psum_prep1", bufs=1, space="PSUM") as psp1:

        # transpose token-major inputs to [din, t]
        def transpose_in(src_ap, tag, eng):
            xt = keep.tile([128, TT], F32, tag=tag)
            for ti, t0 in enumerate(range(0, TT, 128)):
                tl = min(128, TT - t0)
                raw = prep.tile([128, 128], F32, tag="t_raw")
                eng(raw[:tl, :], src_ap[t0 : t0 + tl, :])
                tp = psp.tile([128, 128], F32, tag="t_ps")
                nc.tensor.transpose(tp[:, :tl], raw[:tl, :], ident[:tl, :tl])
                nc.scalar.copy(xt[:, t0 : t0 + tl], tp[:, :tl])
            return xt

        # ---- critical path first: everything the streaming pass needs ----
        xqT = transpose_in(query, "xqT", dma_sync)

        wq = load_const("wq", io["Wq"], [128, 128], dma_sync)
        bq_c = load_const("bq", io["bq"], [128, 1], dma_sync)
        bq_s = consts.tile([128, 1], F32, tag="bq_s")
        nc.scalar.activation(bq_s, bq_c, AF.Copy, scale=SCALE)

        rkw = {}
        small_cols = {}
        for hr in range(H_REL):
            rkw[hr] = load_const(f"rkw{hr}", io["rel_k_w"][hr], [128, DK],
                                 dma_sync)
            small_cols[("rkb", hr)] = load_const(
                f"rkb{hr}", io["rel_k_b"][hr], [DK, 1], dma_sync)
            t = load_const(f"rbias{hr}", io["rel_bias"][0, hr, 0, :], [DK, 1],
                           dma_sync)
            ts_ = consts.tile([DK, 1], F32, tag=f"rbias_s{hr}")
            nc.scalar.activation(ts_, t, AF.Copy, scale=SCALE)
            small_cols[("rbias_s", hr)] = ts_

        rkwT = {}
        for hr in range(H_REL):
            tp = psp.tile([DK, 128], F32, tag="mm_ps")
            nc.tensor.transpose(tp, rkw[hr], ident)
            t = keep.tile([DK, 128], F32, tag=f"rkwT{hr}")
            nc.scalar.copy(t, tp)
            rkwT[hr] = t

        qT = {}
        for h in (H_ABS, H_ABS + 1, 0, 1):  # rel heads first
            qp = psp.tile([DK, TT], F32, tag="mm_ps")
            nc.tensor.matmul(qp, wq[:, DK * h : DK * (h + 1)], xqT)
            t = keep.tile([DK, TT], F32, tag=f"qT{h}")
            nc.scalar.activation(t, qp, AF.Identity,
                                 bias=bq_s[DK * h : DK * (h + 1)], scale=SCALE)
            qT[h] = t

        qrbT = {}
        for hr in range(H_REL):
            t = keep.tile([DK, TT], F32, tag=f"qrbT{hr}")
            nc.vector.tensor_scalar(t, qT[H_ABS + hr],
                                    small_cols[("rbias_s", hr)], None, OP.add)
            qrbT[hr] = t

        # u/c for full blocks: partition = b*64 + (i - i0)
        u_blk = {}
        c_blk = {}
        for hr in range(H_REL):
            for i0 in FULL_I0:
                up = psp1.tile([128, 128], F32, tag="sm_ps")
                cp = psp1.tile([128, 1], F32, tag="sm_psc")
                for b in range(BPC):
                    tsl = slice(b * T + i0, b * T + i0 + HB)
                    nc.tensor.matmul(up[b * HB : (b + 1) * HB, :],
                                     qrbT[hr][:, tsl], rkwT[hr])
                    nc.tensor.matmul(cp[b * HB : (b + 1) * HB, :],
                                     qrbT[hr][:, tsl],
                                     small_cols[("rkb", hr)])
                t = keep.tile([128, 128], BF16, tag=f"ub{hr}_{i0}")
                nc.scalar.copy(t, up)
                u_blk[(hr, i0)] = t
                t = keep.tile([128, 1], F32, tag=f"cb{hr}_{i0}")
                nc.scalar.copy(t, cp)
                c_blk[(hr, i0)] = t

        # u/c for block D (i in [192, 200), both b): 16 rows, row = b*8+(i-192)
        # b1 window first covering [0:16) (8 garbage rows), then b0 [0:8).
        # packed partition p = b*64 + r*8 + jq -> REP = ident16 (x) ones8
        u16 = {}
        c16 = {}
        rep16 = consts.tile([16, 128], BF16, tag="rep16")
        nc.vector.memset(rep16, 0.0)
        rep_view = bass.AP(tensor=rep16.tensor, offset=rep16.offset,
                           ap=[[128, 16], [8, 16], [1, 8]])
        nc.vector.tensor_copy(
            rep_view,
            identb[:16, :16].unsqueeze(2).broadcast_to([16, 16, 8]))
        for hr in range(H_REL):
            up = psp1.tile([16, 128], F32, tag="sm_ps16")
            cp = psp1.tile([16, 1], F32, tag="sm_psc16")
            nc.tensor.matmul(up[0:16, :], qrbT[hr][:, TT - 16 : TT], rkwT[hr])
            nc.tensor.matmul(up[0:8, :], qrbT[hr][:, 192:200], rkwT[hr])
            nc.tensor.matmul(cp[0:16, :], qrbT[hr][:, TT - 16 : TT],
                             small_cols[("rkb", hr)])
            nc.tensor.matmul(cp[0:8, :], qrbT[hr][:, 192:200],
                             small_cols[("rkb", hr)])
            t16 = keep.tile([16, 128], BF16, tag=f"u16_{hr}")
            nc.scalar.copy(t16, up)
            u16[hr] = t16
            tc16 = keep.tile([16, 1], F32, tag=f"c16_{hr}")
            nc.scalar.copy(tc16, cp)
            c16[hr] = tc16
        # replicate u16 8x along partitions: u_rep[jq*16 + r] = u16[r]
        u_rep = {}
        for hr in range(H_REL):
            urp = psp1.tile([128, 128], F32, tag="sm_ps")
            nc.tensor.matmul(urp, rep16, u16[hr])
            t = keep.tile([128, 128], BF16, tag=f"urep{hr}")
            nc.scalar.copy(t, urp)
            u_rep[hr] = t

        # ---- rest of prep (scalar ring) ----
        xkT = transpose_in(key, "xkT", dma_scalar)
        xvT = transpose_in(value, "xvT", dma_scalar)

        wk = load_const("wk", io["Wk"], [128, 128])
        wv = load_const("wv", io["Wv"], [128, 128])
        wo = load_const("wo", io["Wo"], [128, 128])
        bk_c = load_const("bk", io["bk"], [128, 1])
        bv_b = consts.tile([128, 128], F32, tag="bv_b")
        bv_ap = io["bv"]
        dma_scalar(bv_b, bass.AP(tensor=bv_ap.tensor, offset=bv_ap.offset,
                                 ap=[[0, 128]] + bv_ap.ap))
        bo_b = consts.tile([128, 128], F32, tag="bo_b")
        bo_ap = io["bo"]
        dma_scalar(bo_b, bass.AP(tensor=bo_ap.tensor, offset=bo_ap.offset,
                                 ap=[[0, 128]] + bo_ap.ap))

        abs_w = {}
        for hh in range(H_ABS):
            abs_w[("aqw", hh)] = load_const(f"aqw{hh}", io["abs_q_w"][hh], [128, DK])
            abs_w[("akw", hh)] = load_const(f"akw{hh}", io["abs_k_w"][hh], [128, DK])
            small_cols[("akb", hh)] = load_const(
                f"akb{hh}", io["abs_k_b"][hh], [DK, 1])
            t = load_const(f"aqb{hh}", io["abs_q_b"][hh], [DK, 1])
            ts_ = consts.tile([DK, 1], F32, tag=f"aqb_s{hh}")
            nc.scalar.activation(ts_, t, AF.Copy, scale=SCALE)
            small_cols[("aqb_s", hh)] = ts_

        kT = {}
        for h in range(H):
            kp = psp.tile([DK, TT], F32, tag="mm_ps")
            nc.tensor.matmul(kp, wk[:, DK * h : DK * (h + 1)], xkT)
            t = keep.tile([DK, TT], F32, tag=f"kT{h}")
            nc.scalar.activation(t, kp, AF.Identity,
                                 bias=bk_c[DK * h : DK * (h + 1)])
            kT[h] = t

        vb = {}
        for b in range(BPC):
            for jb, (j0, jl) in enumerate(IBLOCKS):
                vp = psp1.tile([128, 128], F32, tag="sm_ps")
                nc.tensor.matmul(vp[:jl, :], xvT[:, b * T + j0 : b * T + j0 + jl], wv)
                t = keep.tile([128, 128], BF16, tag=f"v{b}_{jb}")
                nc.vector.tensor_add(t[:jl, :], vp[:jl, :], bv_b[:jl, :])
                vb[(b, jb)] = t

        qaT = {}
        kaT = {}
        for hh in range(H_ABS):
            akT = transpose_in(absk[hh].flatten_outer_dims(), f"akT{hh}",
                               dma_scalar)
            pp = psp.tile([DK, TT], F32, tag="mm_ps")
            nc.tensor.matmul(pp, abs_w[("aqw", hh)], akT)
            pqT = prep.tile([DK, TT], F32, tag="pqT")
            nc.scalar.activation(pqT, pp, AF.Identity,
                                 bias=small_cols[("aqb_s", hh)], scale=SCALE)
            t = keep.tile([DK, TT], F32, tag=f"qaT{hh}")
            nc.vector.tensor_add(t, qT[hh], pqT)
            qaT[hh] = t

            pp2 = psp.tile([DK, TT], F32, tag="mm_ps")
            nc.tensor.matmul(pp2, abs_w[("akw", hh)], akT)
            pkT = prep.tile([DK, TT], F32, tag="pqT")
            nc.scalar.activation(pkT, pp2, AF.Identity,
                                 bias=small_cols[("akb", hh)])
            t = keep.tile([DK, TT], F32, tag=f"kaT{hh}")
            nc.vector.tensor_add(t, kT[hh], pkT)
            kaT[hh] = t

        # mask tiles: (b, ib) blocks for abs heads
        mb_abs = {}
        for b in range(BPC):
            for ib, (i0, il) in enumerate(IBLOCKS):
                mi = prep.tile([128, T], I32, tag="m_i32")
                dma_scalar(mi[:il, :], mask[b, 0, i0 : i0 + il, :])
                t = keep.tile([128, T], F32, tag=f"mb{b}_{ib}")
                nc.vector.tensor_scalar(t[:il, :], mi[:il, :], 1e9, -1e9,
                                        OP.mult, OP.add)
                mb_abs[(b, ib)] = t

        # mask tiles for stream blocks: partition = b*64 + (i - i0)
        mb_blk = {}
        for i0 in FULL_I0:
            mi = prep.tile([128, T], I32, tag="ms_i32")
            for b in range(BPC):
                dma_scalar(mi[b * HB : (b + 1) * HB, :],
                           mask[b, 0, i0 : i0 + HB, :])
            t = keep.tile([128, T], F32, tag=f"mbs{i0}")
            nc.vector.tensor_scalar(t, mi, 1e9, -1e9, OP.mult, OP.add)
            mb_blk[i0] = t
        mi = prep.tile([16, T], I32, tag="ms_i32l")
        for b in range(BPC):
            dma_scalar(mi[b * DN : (b + 1) * DN, :], mask[b, 0, 192:T, :])
        mb16 = keep.tile([16, T], F32, tag="mb16")
        nc.vector.tensor_scalar(mb16, mi, 1e9, -1e9, OP.mult, OP.add)

    # ---------------- main phase ----------------
    stream = ctx.enter_context(tc.tile_pool(name="stream", bufs=5))
    wpool = ctx.enter_context(tc.tile_pool(name="wpool", bufs=3))
    tree = ctx.enter_context(tc.tile_pool(name="tree", bufs=2))
    s2pool = ctx.enter_context(tc.tile_pool(name="s2pool", bufs=2))
    sm = ctx.enter_context(tc.tile_pool(name="sm", bufs=2))
    ps_s1 = ctx.enter_context(tc.tile_pool(name="ps_s1", bufs=2, space="PSUM"))
    ps_tp = ctx.enter_context(tc.tile_pool(name="ps_tp", bufs=2, space="PSUM"))
    ps_x = ctx.enter_context(tc.tile_pool(name="ps_x", bufs=1, space="PSUM"))
    ps_tail = ctx.enter_context(tc.tile_pool(name="ps_tail", bufs=1, space="PSUM"))

    chunk_n = [0]

    def stream_chunk(dram_ap, s2t, ub, jslice, jcw):
        """One [128, jcw, 128] chunk: dma, mult by ub, tree-reduce over d."""
        rk = stream.tile([128, JC, 128], BF16, tag="rk")
        chunk_n[0] += 1
        dma_eng = nc.sync if chunk_n[0] % 2 == 0 else nc.scalar
        dma_eng.dma_start(rk[:, :jcw, :], dram_ap)
        w = wpool.tile([128, JC, 128], BF16, tag="w")
        nc.vector.tensor_tensor(
            w[:, :jcw, :], rk[:, :jcw, :],
            ub.unsqueeze(1).broadcast_to([128, jcw, 128]), op=OP.mult)
        cur = w
        width = 64
        while width >= 2:
            nxt = tree.tile([128, JC, width], BF16, tag=f"L{width}")
            nc.vector.tensor_add(nxt[:, :jcw, :], cur[:, :jcw, 0:width],
                                 cur[:, :jcw, width : 2 * width])
            cur = nxt
            width //= 2
        nc.vector.tensor_add(s2t[:, jslice], cur[:, :jcw, 0],
                             cur[:, :jcw, 1])

    # PSUM x accumulator: one bank, columns (b*2+ib)*128 + h*DK per head
    x_all = ps_x.tile([128, 512], F32, tag="x_all", name="x_all")

    def x_col(b, ib, h):
        return (b * 2 + ib) * 128 + DK * h

    def softmax_pv(st, rows, segs, h_of_seg):
        """exp + rowsum + normalize + transpose/pv for one score block.
        st: [rows, T] logits (SBUF). segs: [(off, ln, b, i0)]."""
        p = sm.tile([128, T], BF16, tag="p")
        rsum = sm.tile([128, 1], F32, tag="rsum")
        nc.scalar.activation(p[:rows, :], st[:rows, :], AF.Exp,
                             accum_out=rsum[:rows])
        rcp = sm.tile([128, 1], F32, tag="rcp")
        nc.vector.reciprocal(rcp[:rows], rsum[:rows])
        pn = sm.tile([128, T], BF16, tag="pn")
        nc.scalar.activation(pn[:rows, :], p[:rows, :], AF.Copy,
                             scale=rcp[:rows])
        for (off, ln, b, i0), h in zip(segs, h_of_seg):
            ib = 0 if i0 < 128 else 1
            xoff = i0 - ib * 128
            xc = x_col(b, ib, h)
            for jb, (j0, jl) in enumerate(IBLOCKS):
                al = (off // 64) * 64  # 64-aligned covering slice for PE read
                ln_c = off + ln - al
                tp = ps_tp.tile([128, 128], BF16, tag="tp")
                nc.tensor.transpose(tp[:jl, :ln_c],
                                    pn[al : off + ln, j0 : j0 + jl],
                                    identb[al : off + ln, al : off + ln])
                pT = sm.tile([128, 128], BF16, tag="pT")
                nc.scalar.copy(pT[:jl, :ln_c], tp[:jl, :ln_c])
                nc.tensor.matmul(x_all[xoff : xoff + ln, xc : xc + DK],
                                 pT[:jl, off - al : off - al + ln],
                                 vb[(b, jb)][:jl, DK * h : DK * (h + 1)],
                                 start=(jb == 0), stop=(jb == 1))

    # ---- abs-head scores (independent of the stream; emitted first) ----
    for b in range(BPC):
        for ib, (i0, il) in enumerate(IBLOCKS):
            tsl = slice(b * T + i0, b * T + i0 + il)
            for h in range(H_ABS):
                s1 = ps_s1.tile([128, T], F32, tag="s1")
                nc.tensor.matmul(s1[:il, :], qaT[h][:, tsl],
                                 kaT[h][:, b * T : (b + 1) * T])
                st = sm.tile([128, T], F32, tag="st")
                nc.vector.tensor_add(st[:il, :], s1[:il, :],
                                     mb_abs[(b, ib)][:il, :])
                softmax_pv(st, il, [(0, il, b, i0)], [h])

    # ---- the stream + rel scores ----
    def rel_scores_full(hr, i0, s2t):
        h = H_ABS + hr
        s1 = ps_s1.tile([128, T], F32, tag="s1")
        for b in range(BPC):
            nc.tensor.matmul(s1[b * HB : (b + 1) * HB, :],
                             qT[h][:, b * T + i0 : b * T + i0 + HB],
                             kT[h][:, b * T : (b + 1) * T])
        st = sm.tile([128, T], F32, tag="st")
        nc.vector.scalar_tensor_tensor(st, s1, c_blk[(hr, i0)], s2t,
                                       op0=OP.add, op1=OP.add)
        nc.vector.tensor_add(st, st, mb_blk[i0])
        softmax_pv(st, 128,
                   [(b * HB, HB, b, i0) for b in range(BPC)], [h, h])

    rel_stride = T * D  # row stride in rel_kernel elements
    for hr in range(H_REL):
        base = relk[hr]  # [2, 200, 200, 128] -> b, i, j, d
        for i0 in FULL_I0:
            s2t = s2pool.tile([128, T], F32, tag=f"s2_{hr}_{i0}",
                              name=f"s2_{hr}_{i0}")
            for jc0 in range(0, T, JC):
                # partition = (b in 2, i-i0 in 64); free = (j in 50, d)
                ap = bass.AP(
                    tensor=base.tensor,
                    offset=base.offset + i0 * rel_stride + jc0 * D,
                    ap=[[T * rel_stride, BPC], [rel_stride, HB],
                        [D, JC], [1, D]])
                stream_chunk(ap, s2t, u_blk[(hr, i0)],
                             slice(jc0, jc0 + JC), JC)
            rel_scores_full(hr, i0, s2t)

        # block D: partition p = b*64 + r*8 + jq; free = (j' 25, d).
        # r and jq strides merge: r-stride (T*D) = 8 * jq-stride (DJ*D).
        s2p = s2pool.tile([128, DJ], F32, tag=f"s2p_{hr}", name=f"s2p_{hr}")
        ap = bass.AP(
            tensor=base.tensor,
            offset=base.offset + 192 * rel_stride,
            ap=[[T * rel_stride, BPC], [DJ * D, DN * 8], [1, DJ * D]])
        stream_chunk(ap, s2p, u_rep[hr], slice(0, DJ), DJ)
        # unpack: s2_16[row, jq*25+j'] = s2p[8*row + jq, j']
        # lhsT for jq = ident columns [jq::8][:16] (stride-8 column view)
        s2l_ps = ps_tail.tile([16, T], F32, tag="s2l")
        for jq in range(8):
            e_jq = bass.AP(tensor=ident.tensor, offset=ident.offset + jq,
                           ap=[[128, 128], [8, 16]])
            nc.tensor.matmul(s2l_ps[:, jq * DJ : (jq + 1) * DJ],
                             e_jq, s2p[:, 0:DJ], start=True, stop=True)
        s2_16 = sm.tile([16, T], F32, tag="s2_16")
        nc.scalar.copy(s2_16, s2l_ps)

        # block D scores: 16 rows, row = b*8 + (i-192)
        h = H_ABS + hr
        s1 = ps_s1.tile([128, T], F32, tag="s1")
        nc.tensor.matmul(s1[0:16, :], qT[h][:, TT - 16 : TT],
                         kT[h][:, T : 2 * T])
        nc.tensor.matmul(s1[0:8, :], qT[h][:, 192:200], kT[h][:, 0:T])
        st = sm.tile([16, T], F32, tag="std")
        nc.vector.scalar_tensor_tensor(st, s1[0:16, :], c16[hr], s2_16,
                                       op0=OP.add, op1=OP.add)
        nc.vector.tensor_add(st, st, mb16)
        softmax_pv(st, 16, [(b * DN, DN, b, 192) for b in range(BPC)],
                   [h, h])

    # ---- output: x scaling already folded; project ----
    for b in range(BPC):
        for ib, (i0, il) in enumerate(IBLOCKS):
            xc = (b * 2 + ib) * 128
            x_sb = sm.tile([128, 128], F32, tag="x_sb")
            nc.scalar.copy(x_sb[:il, :], x_all[:il, xc : xc + 128])
            xT_ps = ps_tail.tile([128, 128], F32, tag="tail3")
            nc.tensor.transpose(xT_ps[:, :il], x_sb[:il, :], ident[:il, :il])
            xT_sb = sm.tile([128, 128], F32, tag="xT_sb")
            nc.scalar.copy(xT_sb[:, :il], xT_ps[:, :il])
            y_ps = ps_tail.tile([128, 128], F32, tag="tail3")
            nc.tensor.matmul(y_ps[:il, :], xT_sb[:, :il], wo)
            y_sb = sm.tile([128, 128], F32, tag="y_sb")
            nc.vector.tensor_add(y_sb[:il, :], y_ps[:il, :], bo_b[:il, :])
            nc.sync.dma_start(out[b, i0 : i0 + il, :], y_sb[:il, :])


def build_nc():
    nc = bacc.Bacc(trn_type="TRN2")
    io = {}
    io["query"] = nc.dram_tensor("query", [BPC, T, D], F32, kind="ExternalInput").ap()
    io["key"] = nc.dram_tensor("key", [BPC, T, D], F32, kind="ExternalInput").ap()
    io["value"] = nc.dram_tensor("value", [BPC, T, D], F32, kind="ExternalInput").ap()
    io["mask"] = nc.dram_tensor("mask", [BPC, 1, T, T], I32, kind="ExternalInput").ap()
    io["rel_kernel"] = nc.dram_tensor(
        "rel_kernel", [H_REL, BPC, T, T, D], BF16, kind="ExternalInput"
    ).ap()
    io["abs_kernel"] = nc.dram_tensor(
        "abs_kernel", [H_ABS, BPC, T, D], F32, kind="ExternalInput"
    ).ap()
    for nm, shape in [
        ("Wq", [D, D]), ("bq", [D]), ("Wk", [D, D]), ("bk", [D]),
        ("Wv", [D, D]), ("bv", [D]),
        ("abs_q_w", [H_ABS, D, DK]), ("abs_q_b", [H_ABS, DK]),
        ("abs_k_w", [H_ABS, D, DK]), ("abs_k_b", [H_ABS, DK]),
        ("rel_k_w", [H_REL, D, DK]), ("rel_k_b", [H_REL, DK]),
        ("rel_bias", [1, H_REL, 1, DK]),
        ("Wo", [D, D]), ("bo", [D]),
    ]:
        io[nm] = nc.dram_tensor(nm, shape, F32, kind="ExternalInput").ap()
    io["out"] = nc.dram_tensor("out", [BPC, T, D], F32, kind="ExternalOutput").ap()

    with tile.TileContext(nc) as tc:
        with ExitStack() as ctx:
            build_kernel(ctx, tc, io)
    nc.compile()
    return nc


_NC_CACHE = None


def _get_nc():
    global _NC_CACHE
    if _NC_CACHE is None:
        _NC_CACHE = build_nc()
    return _NC_CACHE


def make_in_maps(inputs):
    """Shard full inputs into per-core input maps."""
    f32 = np.float32
    weights = {
        nm: np.ascontiguousarray(np.asarray(inputs[nm], dtype=f32))
        for nm in ["Wq", "bq", "Wk", "bk", "Wv", "bv", "abs_q_w", "abs_q_b",
                   "abs_k_w", "abs_k_b", "rel_k_w", "rel_k_b", "rel_bias",
                   "Wo", "bo"]
    }
    query = np.asarray(inputs["query"], dtype=f32)
    key = np.asarray(inputs["key"], dtype=f32)
    value = np.asarray(inputs["value"], dtype=f32)
    mask = np.asarray(inputs["mask"], dtype=np.int32)
    import ml_dtypes
    relk = np.asarray(inputs["rel_kernel"], dtype=f32).astype(ml_dtypes.bfloat16)
    absk = np.asarray(inputs["abs_kernel"], dtype=f32)

    in_maps = []
    for c in range(N_CORES):
        bs = slice(c * BPC, (c + 1) * BPC)
        m = dict(weights)
        m["query"] = np.ascontiguousarray(query[bs])
        m["key"] = np.ascontiguousarray(key[bs])
        m["value"] = np.ascontiguousarray(value[bs])
        m["mask"] = np.ascontiguousarray(mask[bs])
        m["rel_kernel"] = np.ascontiguousarray(relk[:, bs])
        m["abs_kernel"] = np.ascontiguousarray(absk[:, bs])
        in_maps.append(m)
    return in_maps


def kernel(**inputs) -> np.ndarray:
    nc = _get_nc()
    in_maps = make_in_maps(inputs)
    res = run_bass_kernel_spmd(nc, in_maps, core_ids=list(range(N_CORES)))
    return np.concatenate([r["out"] for r in res.results], axis=0)


if __name__ == "__main__":
    nc = build_nc()
    print("built ok")


# revision 31
# speedup vs baseline: 2.9768x; 2.9768x over previous
"""Trainium2 Bass kernel for nn_Attention_12463995093474 (sparse_attention).

Math (reference):
  q/k/v = content linears; 2 absolute heads, 2 relative heads (DK=32).
  abs:  scores = (Xq_a + abs_kernel@abs_q_w) @ (Xk_a + abs_kernel@abs_k_w)^T
  rel:  scores = Xq_r @ Xk_r^T + (Xq_r + rel_bias) . (rel_kernel@rel_k_w + rel_k_b)
  softmax(mask) @ v -> out linear.

Key optimization: the dominant term
    sum_o (Xq_r+rel_bias)[i,o] * (sum_d rel_kernel[i,j,d] rel_k_w[d,o] + rel_k_b[o])
is reassociated to
    sum_d rel_kernel[i,j,d] * u[i,d] + c[i],
      u = rel_k_w @ (Xq_r+rel_bias)^T,  c = (Xq_r+rel_bias) @ rel_k_b
which turns a 21-GFLOP tensor contraction into a single streaming pass over
rel_kernel (655 MB) in bf16 on the Vector engine: elementwise multiply by u
(free-dim-broadcast AP) + pairwise tree reduction over d.

DVE op cost is free-dim-size bound, so every streaming chunk must use all
128 partitions. Per rel head hr, the 2x200 (batch, i) rows are packed as:
  blocks A/B/C: partition = (b in 2) x (i-halfblock of 64) -> 128 rows
  block D (i in [192,200)): partition = (jq in 8) x (b in 2) x (i in 8),
    with j split in 8 chunks of 25; uses a jq-replicated u operand, and its
    s2 result is unpacked back to row-major with 8 identity-slice matmuls.
All PE writes land on 0/64 partition bases (hardware requires 32-aligned).

Softmax skips the max-subtraction (logits are O(1); masked entries are -1e9
and exp underflows to exactly 0); p normalized on the Scalar engine in bf16.

Sharding: data-parallel over batch, B=16 -> 2 batches per core on 8 cores.
"""

import numpy as np
from contextlib import ExitStack

import concourse.bass as bass
import concourse.bacc as bacc
import concourse.tile as tile
from concourse import mybir
from concourse.masks import make_identity
from concourse.bass_utils import run_bass_kernel_spmd

B, T, D = 16, 200, 128
H_ABS, H_REL, H, DK = 2, 2, 4, 32
N_CORES = 8
BPC = B // N_CORES  # batches per core
SCALE = 1.0 / float(DK) ** 0.5
JC = 50  # j-chunk for full streaming blocks (T % JC == 0)
TT = BPC * T  # tokens per core (400)

F32 = mybir.dt.float32
BF16 = mybir.dt.bfloat16
I32 = mybir.dt.int32
AX = mybir.AxisListType
OP = mybir.AluOpType
AF = mybir.ActivationFunctionType

# abs-head i-blocks per batch: (start, len)
IBLOCKS = [(0, 128), (128, T - 128)]
# full stream blocks per hr: i-halfblocks of 64, two batches stacked
HB = 64
FULL_I0 = [0, 64, 128]
DN = T - 192  # 8 leftover i-rows -> packed block D
DJ = T // 8   # 25: j-chunk for block D


def build_kernel(ctx: ExitStack, tc: tile.TileContext, io: dict):
    nc = tc.nc

    query = io["query"].flatten_outer_dims()  # [400, 128]
    key = io["key"].flatten_outer_dims()
    value = io["value"].flatten_outer_dims()
    mask = io["mask"]          # [2, 1, 200, 200] i32
    relk = io["rel_kernel"]    # [2, 2, 200, 200, 128] bf16
    absk = io["abs_kernel"]    # [2, 2, 200, 128]
    out = io["out"]            # [2, 200, 128]

    consts = ctx.enter_context(tc.tile_pool(name="consts", bufs=1))
    prep = ctx.enter_context(tc.tile_pool(name="prep", bufs=2))
    keep = ctx.enter_context(tc.tile_pool(name="keep", bufs=1))

    def dma_sync(out_ap, in_ap):
        return nc.sync.dma_start(out_ap, in_ap)

    def dma_scalar(out_ap, in_ap):
        return nc.scalar.dma_start(out_ap, in_ap)

    ident = consts.tile([128, 128], F32, tag="ident")
    make_identity(nc, ident)
    identb = consts.tile([128, 128], BF16, tag="identb")
    nc.gpsimd.tensor_copy(identb, ident)

    def load_const(name, ap, shape, eng=dma_scalar):
        t = consts.tile(shape, F32, tag=name)
        eng(t, ap)
        return t

    with tc.tile_pool(name="psum_prep", bufs=2, space="PSUM") as psp, \
         tc.tile_pool(name="psum_prep1", bufs=1, space="PSUM") as psp1:

        # transpose token-major inputs to [din, t]
        def transpose_in(src_ap, tag, eng):
            xt = keep.tile([128, TT], F32, tag=tag)
            for ti, t0 in enumerate(range(0, TT, 128)):
                tl = min(128, TT - t0)
                raw = prep.tile([128, 128], F32, tag="t_raw")
                eng(raw[:tl, :], src_ap[t0 : t0 + tl, :])
                tp = psp.tile([128, 128], F32, tag="t_ps")
                nc.tensor.transpose(tp[:, :tl], raw[:tl, :], ident[:tl, :tl])
                nc.scalar.copy(xt[:, t0 : t0 + tl], tp[:, :tl])
            return xt

        # ---- critical path first: everything the streaming pass needs ----
        xqT = transpose_in(query, "xqT", dma_sync)

        wq = load_const("wq", io["Wq"], [128, 128], dma_sync)
        bq_c = load_const("bq", io["bq"], [128, 1], dma_sync)
        bq_s = consts.tile([128, 1], F32, tag="bq_s")
        nc.scalar.activation(bq_s, bq_c, AF.Copy, scale=SCALE)

        rkw = {}
        small_cols = {}
        for hr in range(H_REL):
            rkw[hr] = load_const(f"rkw{hr}", io["rel_k_w"][hr], [128, DK],
                                 dma_sync)
            small_cols[("rkb", hr)] = load_const(
                f"rkb{hr}", io["rel_k_b"][hr], [DK, 1], dma_sync)
            t = load_const(f"rbias{hr}", io["rel_bias"][0, hr, 0, :], [DK, 1],
                           dma_sync)
            ts_ = consts.tile([DK, 1], F32, tag=f"rbias_s{hr}")
            nc.scalar.activation(ts_, t, AF.Copy, scale=SCALE)
            small_cols[("rbias_s", hr)] = ts_

        rkwT = {}
        for hr in range(H_REL):
            tp = psp.tile([DK, 128], F32, tag="mm_ps")
            nc.tensor.transpose(tp, rkw[hr], ident)
            t = keep.tile([DK, 128], F32, tag=f"rkwT{hr}")
            nc.scalar.copy(t, tp)
            rkwT[hr] = t

        qT = {}
        for h in (H_ABS, H_ABS + 1, 0, 1):  # rel heads first
            qp = psp.tile([DK, TT], F32, tag="mm_ps")
            nc.tensor.matmul(qp, wq[:, DK * h : DK * (h + 1)], xqT)
            t = keep.tile([DK, TT], F32, tag=f"qT{h}")
            nc.scalar.activation(t, qp, AF.Identity,
                                 bias=bq_s[DK * h : DK * (h + 1)], scale=SCALE)
            qT[h] = t

        qrbT = {}
        for hr in range(H_REL):
            t = keep.tile([DK, TT], F32, tag=f"qrbT{hr}")
            nc.vector.tensor_scalar(t, qT[H_ABS + hr],
                                    small_cols[("rbias_s", hr)], None, OP.add)
            qrbT[hr] = t

        # u/c for full blocks: partition = b*64 + (i - i0)
        u_blk = {}
        c_blk = {}
        for hr in range(H_REL):
            for i0 in FULL_I0:
                up = psp1.tile([128, 128], F32, tag="sm_ps")
                cp = psp1.tile([128, 1], F32, tag="sm_psc")
                for b in range(BPC):
                    tsl = slice(b * T + i0, b * T + i0 + HB)
                    nc.tensor.matmul(up[b * HB : (b + 1) * HB, :],
                                     qrbT[hr][:, tsl], rkwT[hr])
                    nc.tensor.matmul(cp[b * HB : (b + 1) * HB, :],
                                     qrbT[hr][:, tsl],
                                     small_cols[("rkb", hr)])
                t = keep.tile([128, 128], BF16, tag=f"ub{hr}_{i0}")
                nc.scalar.copy(t, up)
                u_blk[(hr, i0)] = t
                t = keep.tile([128, 1], F32, tag=f"cb{hr}_{i0}")
                nc.scalar.copy(t, cp)
                c_blk[(hr, i0)] = t

        # u/c for block D (i in [192, 200), both b): 16 rows, row = b*8+(i-192)
        # b1 window first covering [0:16) (8 garbage rows), then b0 [0:8).
        # packed partition p = b*64 + r*8 + jq -> REP = ident16 (x) ones8
        u16 = {}
        c16 = {}
        rep16 = consts.tile([16, 128], BF16, tag="rep16")
        nc.vector.memset(rep16, 0.0)
        rep_view = bass.AP(tensor=rep16.tensor, offset=rep16.offset,
                           ap=[[128, 16], [8, 16], [1, 8]])
        nc.vector.tensor_copy(
            rep_view,
            identb[:16, :16].unsqueeze(2).broadcast_to([16, 16, 8]))
        for hr in range(H_REL):
            up = psp1.tile([16, 128], F32, tag="sm_ps16")
            cp = psp1.tile([16, 1], F32, tag="sm_psc16")
            nc.tensor.matmul(up[0:16, :], qrbT[hr][:, TT - 16 : TT], rkwT[hr])
            nc.tensor.matmul(up[0:8, :], qrbT[hr][:, 192:200], rkwT[hr])
            nc.tensor.matmul(cp[0:16, :], qrbT[hr][:, TT - 16 : TT],
                             small_cols[("rkb", hr)])
            nc.tensor.matmul(cp[0:8, :], qrbT[hr][:, 192:200],
                             small_cols[("rkb", hr)])
            t16 = keep.tile([16, 128], BF16, tag=f"u16_{hr}")
            nc.scalar.copy(t16, up)
            u16[hr] = t16
            tc16 = keep.tile([16, 1], F32, tag=f"c16_{hr}")
            nc.scalar.copy(tc16, cp)
            c16[hr] = tc16
        # replicate u16 8x along partitions: u_rep[jq*16 + r] = u16[r]
        u_rep = {}
        for hr in range(H_REL):
            urp = psp1.tile([128, 128], F32, tag="sm_ps")
            nc.tensor.matmul(urp, rep16, u16[hr])
            t = keep.tile([128, 128], BF16, tag=f"urep{hr}")
            nc.scalar.copy(t, urp)
            u_rep[hr] = t

        # ---- rest of prep (scalar ring) ----
        xkT = transpose_in(key, "xkT", dma_scalar)
        xvT = transpose_in(value, "xvT", dma_scalar)

        wk = load_const("wk", io["Wk"], [128, 128])
        wv = load_const("wv", io["Wv"], [128, 128])
        wo = load_const("wo", io["Wo"], [128, 128])
        bk_c = load_const("bk", io["bk"], [128, 1])
        bv_b = consts.tile([128, 128], F32, tag="bv_b")
        bv_ap = io["bv"]
        dma_scalar(bv_b, bass.AP(tensor=bv_ap.tensor, offset=bv_ap.offset,
                                 ap=[[0, 128]] + bv_ap.ap))
        bo_b = consts.tile([128, 128], F32, tag="bo_b")
        bo_ap = io["bo"]
        dma_scalar(bo_b, bass.AP(tensor=bo_ap.tensor, offset=bo_ap.offset,
                                 ap=[[0, 128]] + bo_ap.ap))

        abs_w = {}
        for hh in range(H_ABS):
            abs_w[("aqw", hh)] = load_const(f"aqw{hh}", io["abs_q_w"][hh], [128, DK])
            abs_w[("akw", hh)] = load_const(f"akw{hh}", io["abs_k_w"][hh], [128, DK])
            small_cols[("akb", hh)] = load_const(
                f"akb{hh}", io["abs_k_b"][hh], [DK, 1])
            t = load_const(f"aqb{hh}", io["abs_q_b"][hh], [DK, 1])
            ts_ = consts.tile([DK, 1], F32, tag=f"aqb_s{hh}")
            nc.scalar.activation(ts_, t, AF.Copy, scale=SCALE)
            small_cols[("aqb_s", hh)] = ts_

        kT = {}
        for h in range(H):
            kp = psp.tile([DK, TT], F32, tag="mm_ps")
            nc.tensor.matmul(kp, wk[:, DK * h : DK * (h + 1)], xkT)
            t = keep.tile([DK, TT], F32, tag=f"kT{h}")
            nc.scalar.activation(t, kp, AF.Identity,
                                 bias=bk_c[DK * h : DK * (h + 1)])
            kT[h] = t

        vb = {}
        for b in range(BPC):
            for jb, (j0, jl) in enumerate(IBLOCKS):
                vp = psp1.tile([128, 128], F32, tag="sm_ps")
                nc.tensor.matmul(vp[:jl, :], xvT[:, b * T + j0 : b * T + j0 + jl], wv)
                t = keep.tile([128, 128], BF16, tag=f"v{b}_{jb}")
                nc.vector.tensor_add(t[:jl, :], vp[:jl, :], bv_b[:jl, :])
                vb[(b, jb)] = t

        qaT = {}
        kaT = {}
        for hh in range(H_ABS):
            akT = transpose_in(absk[hh].flatten_outer_dims(), f"akT{hh}",
                               dma_scalar)
            pp = psp.tile([DK, TT], F32, tag="mm_ps")
            nc.tensor.matmul(pp, abs_w[("aqw", hh)], akT)
            pqT = prep.tile([DK, TT], F32, tag="pqT")
            nc.scalar.activation(pqT, pp, AF.Identity,
                                 bias=small_cols[("aqb_s", hh)], scale=SCALE)
            t = keep.tile([DK, TT], F32, tag=f"qaT{hh}")
            nc.vector.tensor_add(t, qT[hh], pqT)
            qaT[hh] = t

            pp2 = psp.tile([DK, TT], F32, tag="mm_ps")
            nc.tensor.matmul(pp2, abs_w[("akw", hh)], akT)
            pkT = prep.tile([DK, TT], F32, tag="pqT")
            nc.scalar.activation(pkT, pp2, AF.Identity,
                                 bias=small_cols[("akb", hh)])
            t = keep.tile([DK, TT], F32, tag=f"kaT{hh}")
            nc.vector.tensor_add(t, kT[hh], pkT)
            kaT[hh] = t

        # mask tiles: (b, ib) blocks for abs heads
        mb_abs = {}
        for b in range(BPC):
            for ib, (i0, il) in enumerate(IBLOCKS):
                mi = prep.tile([128, T], I32, tag="m_i32")
                dma_scalar(mi[:il, :], mask[b, 0, i0 : i0 + il, :])
                t = keep.tile([128, T], F32, tag=f"mb{b}_{ib}")
                nc.vector.tensor_scalar(t[:il, :], mi[:il, :], 1e9, -1e9,
                                        OP.mult, OP.add)
                mb_abs[(b, ib)] = t

        # mask tiles for stream blocks: partition = b*64 + (i - i0)
        mb_blk = {}
        for i0 in FULL_I0:
            mi = prep.tile([128, T], I32, tag="ms_i32")
            for b in range(BPC):
                dma_scalar(mi[b * HB : (b + 1) * HB, :],
                           mask[b, 0, i0 : i0 + HB, :])
            t = keep.tile([128, T], F32, tag=f"mbs{i0}")
            nc.vector.tensor_scalar(t, mi, 1e9, -1e9, OP.mult, OP.add)
            mb_blk[i0] = t
        mi = prep.tile([16, T], I32, tag="ms_i32l")
        for b in range(BPC):
            dma_scalar(mi[b * DN : (b + 1) * DN, :], mask[b, 0, 192:T, :])
        mb16 = keep.tile([16, T], F32, tag="mb16")
        nc.vector.tensor_scalar(mb16, mi, 1e9, -1e9, OP.mult, OP.add)

    # ---------------- main phase ----------------
    stream = ctx.enter_context(tc.tile_pool(name="stream", bufs=5))
    wpool = ctx.enter_context(tc.tile_pool(name="wpool", bufs=3))
    tree = ctx.enter_context(tc.tile_pool(name="tree", bufs=2))
    s2pool = ctx.enter_context(tc.tile_pool(name="s2pool", bufs=2))
    sm = ctx.enter_context(tc.tile_pool(name="sm", bufs=2))
    ps_s1 = ctx.enter_context(tc.tile_pool(name="ps_s1", bufs=2, space="PSUM"))
    ps_tp = ctx.enter_context(tc.tile_pool(name="ps_tp", bufs=2, space="PSUM"))
    ps_x = ctx.enter_context(tc.tile_pool(name="ps_x", bufs=1, space="PSUM"))
    ps_tail = ctx.enter_context(tc.tile_pool(name="ps_tail", bufs=1, space="PSUM"))

    chunk_n = [0]

    def stream_chunk(dram_aps, s2t, ub, jslice, jcw):
        """One [128, jcw, 128] chunk: dma halves, mult by ub, d-tree-reduce.
        dram_aps: list of (partition_slice, src_ap) — each src must have a
        single leading partition dim so the DMA spreads across engines."""
        rk = stream.tile([128, JC, 128], BF16, tag="rk")
        chunk_n[0] += 1
        dma_eng = nc.sync if chunk_n[0] % 2 == 0 else nc.scalar
        for psl, src in dram_aps:
            dma_eng.dma_start(rk[psl, :jcw, :], src)
        w = wpool.tile([128, JC, 128], BF16, tag="w")
        nc.vector.tensor_tensor(
            w[:, :jcw, :], rk[:, :jcw, :],
            ub.unsqueeze(1).broadcast_to([128, jcw, 128]), op=OP.mult)
        cur = w
        width = 64
        while width >= 2:
            nxt = tree.tile([128, JC, width], BF16, tag=f"L{width}")
            nc.vector.tensor_add(nxt[:, :jcw, :], cur[:, :jcw, 0:width],
                                 cur[:, :jcw, width : 2 * width])
            cur = nxt
            width //= 2
        nc.vector.tensor_add(s2t[:, jslice], cur[:, :jcw, 0],
                             cur[:, :jcw, 1])

    # PSUM x accumulator: one bank, columns (b*2+ib)*128 + h*DK per head
    x_all = ps_x.tile([128, 512], F32, tag="x_all", name="x_all")

    def x_col(b, ib, h):
        return (b * 2 + ib) * 128 + DK * h

    def softmax_pv(st, rows, segs, h_of_seg):
        """exp + rowsum + normalize + transpose/pv for one score block.
        st: [rows, T] logits (SBUF). segs: [(off, ln, b, i0)]."""
        p = sm.tile([128, T], BF16, tag="p")
        rsum = sm.tile([128, 1], F32, tag="rsum")
        nc.scalar.activation(p[:rows, :], st[:rows, :], AF.Exp,
                             accum_out=rsum[:rows])
        rcp = sm.tile([128, 1], F32, tag="rcp")
        nc.vector.reciprocal(rcp[:rows], rsum[:rows])
        pn = sm.tile([128, T], BF16, tag="pn")
        nc.scalar.activation(pn[:rows, :], p[:rows, :], AF.Copy,
                             scale=rcp[:rows])
        for (off, ln, b, i0), h in zip(segs, h_of_seg):
            ib = 0 if i0 < 128 else 1
            xoff = i0 - ib * 128
            xc = x_col(b, ib, h)
            for jb, (j0, jl) in enumerate(IBLOCKS):
                al = (off // 64) * 64  # 64-aligned covering slice for PE read
                ln_c = off + ln - al
                tp = ps_tp.tile([128, 128], BF16, tag="tp")
                nc.tensor.transpose(tp[:jl, :ln_c],
                                    pn[al : off + ln, j0 : j0 + jl],
                                    identb[al : off + ln, al : off + ln])
                pT = sm.tile([128, 128], BF16, tag="pT")
                nc.scalar.copy(pT[:jl, :ln_c], tp[:jl, :ln_c])
                nc.tensor.matmul(x_all[xoff : xoff + ln, xc : xc + DK],
                                 pT[:jl, off - al : off - al + ln],
                                 vb[(b, jb)][:jl, DK * h : DK * (h + 1)],
                                 start=(jb == 0), stop=(jb == 1))

    # ---- abs-head scores (independent of the stream; emitted first) ----
    for b in range(BPC):
        for ib, (i0, il) in enumerate(IBLOCKS):
            tsl = slice(b * T + i0, b * T + i0 + il)
            for h in range(H_ABS):
                s1 = ps_s1.tile([128, T], F32, tag="s1")
                nc.tensor.matmul(s1[:il, :], qaT[h][:, tsl],
                                 kaT[h][:, b * T : (b + 1) * T])
                st = sm.tile([128, T], F32, tag="st")
                nc.vector.tensor_add(st[:il, :], s1[:il, :],
                                     mb_abs[(b, ib)][:il, :])
                softmax_pv(st, il, [(0, il, b, i0)], [h])

    # ---- the stream + rel scores ----
    def rel_scores_full(hr, i0, s2t):
        h = H_ABS + hr
        s1 = ps_s1.tile([128, T], F32, tag="s1")
        for b in range(BPC):
            nc.tensor.matmul(s1[b * HB : (b + 1) * HB, :],
                             qT[h][:, b * T + i0 : b * T + i0 + HB],
                             kT[h][:, b * T : (b + 1) * T])
        st = sm.tile([128, T], F32, tag="st")
        nc.vector.scalar_tensor_tensor(st, s1, c_blk[(hr, i0)], s2t,
                                       op0=OP.add, op1=OP.add)
        nc.vector.tensor_add(st, st, mb_blk[i0])
        softmax_pv(st, 128,
                   [(b * HB, HB, b, i0) for b in range(BPC)], [h, h])

    rel_stride = T * D  # row stride in rel_kernel elements
    for hr in range(H_REL):
        base = relk[hr]  # [2, 200, 200, 128] -> b, i, j, d
        for i0 in FULL_I0:
            s2t = s2pool.tile([128, T], F32, tag=f"s2_{hr}_{i0}",
                              name=f"s2_{hr}_{i0}")
            for jc0 in range(0, T, JC):
                # partition = (b in 2, i-i0 in 64); free = (j in 50, d)
                aps = []
                for b in range(BPC):
                    src = bass.AP(
                        tensor=base.tensor,
                        offset=(base.offset + (b * T + i0) * rel_stride
                                + jc0 * D),
                        ap=[[rel_stride, HB], [D, JC], [1, D]])
                    aps.append((slice(b * HB, (b + 1) * HB), src))
                stream_chunk(aps, s2t, u_blk[(hr, i0)],
                             slice(jc0, jc0 + JC), JC)
            rel_scores_full(hr, i0, s2t)

        # block D: partition p = b*64 + r*8 + jq; free = (j' 25, d).
        # r and jq strides merge: r-stride (T*D) = 8 * jq-stride (DJ*D).
        s2p = s2pool.tile([128, DJ], F32, tag=f"s2p_{hr}", name=f"s2p_{hr}")
        aps = []
        for b in range(BPC):
            src = bass.AP(
                tensor=base.tensor,
                offset=base.offset + (b * T + 192) * rel_stride,
                ap=[[DJ * D, DN * 8], [1, DJ * D]])
            aps.append((slice(b * HB, (b + 1) * HB), src))
        stream_chunk(aps, s2p, u_rep[hr], slice(0, DJ), DJ)
        # unpack: s2_16[row, jq*25+j'] = s2p[8*row + jq, j']
        # lhsT for jq = ident columns [jq::8][:16] (stride-8 column view)
        s2l_ps = ps_tail.tile([16, T], F32, tag="s2l")
        for jq in range(8):
            e_jq = bass.AP(tensor=ident.tensor, offset=ident.offset + jq,
                           ap=[[128, 128], [8, 16]])
            nc.tensor.matmul(s2l_ps[:, jq * DJ : (jq + 1) * DJ],
                             e_jq, s2p[:, 0:DJ], start=True, stop=True)
        s2_16 = sm.tile([16, T], F32, tag="s2_16")
        nc.scalar.copy(s2_16, s2l_ps)

        # block D scores: 16 rows, row = b*8 + (i-192)
        h = H_ABS + hr
        s1 = ps_s1.tile([128, T], F32, tag="s1")
        nc.tensor.matmul(s1[0:16, :], qT[h][:, TT - 16 : TT],
                         kT[h][:, T : 2 * T])
        nc.tensor.matmul(s1[0:8, :], qT[h][:, 192:200], kT[h][:, 0:T])
        st = sm.tile([16, T], F32, tag="std")
        nc.vector.scalar_tensor_tensor(st, s1[0:16, :], c16[hr], s2_16,
                                       op0=OP.add, op1=OP.add)
        nc.vector.tensor_add(st, st, mb16)
        softmax_pv(st, 16, [(b * DN, DN, b, 192) for b in range(BPC)],
                   [h, h])

    # ---- output: x scaling already folded; project ----
    for b in range(BPC):
        for ib, (i0, il) in enumerate(IBLOCKS):
            xc = (b * 2 + ib) * 128
            x_sb = sm.tile([128, 128], F32, tag="x_sb")
            nc.scalar.copy(x_sb[:il, :], x_all[:il, xc : xc + 128])
            xT_ps = ps_tail.tile([128, 128], F32, tag="tail3")
            nc.tensor.transpose(xT_ps[:, :il], x_sb[:il, :], ident[:il, :il])
            xT_sb = sm.tile([128, 128], F32, tag="xT_sb")
            nc.scalar.copy(xT_sb[:, :il], xT_ps[:, :il])
            y_ps = ps_tail.tile([128, 128], F32, tag="tail3")
            nc.tensor.matmul(y_ps[:il, :], xT_sb[:, :il], wo)
            y_sb = sm.tile([128, 128], F32, tag="y_sb")
            nc.vector.tensor_add(y_sb[:il, :], y_ps[:il, :], bo_b[:il, :])
            nc.sync.dma_start(out[b, i0 : i0 + il, :], y_sb[:il, :])


def build_nc():
    nc = bacc.Bacc(trn_type="TRN2")
    io = {}
    io["query"] = nc.dram_tensor("query", [BPC, T, D], F32, kind="ExternalInput").ap()
    io["key"] = nc.dram_tensor("key", [BPC, T, D], F32, kind="ExternalInput").ap()
    io["value"] = nc.dram_tensor("value", [BPC, T, D], F32, kind="ExternalInput").ap()
    io["mask"] = nc.dram_tensor("mask", [BPC, 1, T, T], I32, kind="ExternalInput").ap()
    io["rel_kernel"] = nc.dram_tensor(
        "rel_kernel", [H_REL, BPC, T, T, D], BF16, kind="ExternalInput"
    ).ap()
    io["abs_kernel"] = nc.dram_tensor(
        "abs_kernel", [H_ABS, BPC, T, D], F32, kind="ExternalInput"
    ).ap()
    for nm, shape in [
        ("Wq", [D, D]), ("bq", [D]), ("Wk", [D, D]), ("bk", [D]),
        ("Wv", [D, D]), ("bv", [D]),
        ("abs_q_w", [H_ABS, D, DK]), ("abs_q_b", [H_ABS, DK]),
        ("abs_k_w", [H_ABS, D, DK]), ("abs_k_b", [H_ABS, DK]),
        ("rel_k_w", [H_REL, D, DK]), ("rel_k_b", [H_REL, DK]),
        ("rel_bias", [1, H_REL, 1, DK]),
        ("Wo", [D, D]), ("bo", [D]),
    ]:
        io[nm] = nc.dram_tensor(nm, shape, F32, kind="ExternalInput").ap()
    io["out"] = nc.dram_tensor("out", [BPC, T, D], F32, kind="ExternalOutput").ap()

    with tile.TileContext(nc) as tc:
        with ExitStack() as ctx:
            build_kernel(ctx, tc, io)
    nc.compile()
    return nc


_NC_CACHE = None


def _get_nc():
    global _NC_CACHE
    if _NC_CACHE is None:
        _NC_CACHE = build_nc()
    return _NC_CACHE


def make_in_maps(inputs):
    """Shard full inputs into per-core input maps."""
    f32 = np.float32
    weights = {
        nm: np.ascontiguousarray(np.asarray(inputs[nm], dtype=f32))
        for nm in ["Wq", "bq", "Wk", "bk", "Wv", "bv", "abs_q_w", "abs_q_b",
                   "abs_k_w", "abs_k_b", "rel_k_w", "rel_k_b", "rel_bias",
                   "Wo", "bo"]
    }
    query = np.asarray(inputs["query"], dtype=f32)
    key = np.asarray(inputs["key"], dtype=f32)
    value = np.asarray(inputs["value"], dtype=f32)
    mask = np.asarray(inputs["mask"], dtype=np.int32)
    import ml_dtypes
    relk = np.asarray(inputs["rel_kernel"], dtype=f32).astype(ml_dtypes.bfloat16)
    absk = np.asarray(inputs["abs_kernel"], dtype=f32)

    in_maps = []
    for c in range(N_CORES):
        bs = slice(c * BPC, (c + 1) * BPC)
        m = dict(weights)
        m["query"] = np.ascontiguousarray(query[bs])
        m["key"] = np.ascontiguousarray(key[bs])
        m["value"] = np.ascontiguousarray(value[bs])
        m["mask"] = np.ascontiguousarray(mask[bs])
        m["rel_kernel"] = np.ascontiguousarray(relk[:, bs])
        m["abs_kernel"] = np.ascontiguousarray(absk[:, bs])
        in_maps.append(m)
    return in_maps


def kernel(**inputs) -> np.ndarray:
    nc = _get_nc()
    in_maps = make_in_maps(inputs)
    res = run_bass_kernel_spmd(nc, in_maps, core_ids=list(range(N_CORES)))
    return np.concatenate([r["out"] for r in res.results], axis=0)


if __name__ == "__main__":
    nc = build_nc()
    print("built ok")


# revision 39
# speedup vs baseline: 3.0026x; 1.0086x over previous
"""Trainium2 Bass kernel for nn_Attention_12463995093474 (sparse_attention).

Math (reference):
  q/k/v = content linears; 2 absolute heads, 2 relative heads (DK=32).
  abs:  scores = (Xq_a + abs_kernel@abs_q_w) @ (Xk_a + abs_kernel@abs_k_w)^T
  rel:  scores = Xq_r @ Xk_r^T + (Xq_r + rel_bias) . (rel_kernel@rel_k_w + rel_k_b)
  softmax(mask) @ v -> out linear.

Key optimization: the dominant term
    sum_o (Xq_r+rel_bias)[i,o] * (sum_d rel_kernel[i,j,d] rel_k_w[d,o] + rel_k_b[o])
is reassociated to
    sum_d rel_kernel[i,j,d] * u[i,d] + c[i],
      u = rel_k_w @ (Xq_r+rel_bias)^T,  c = (Xq_r+rel_bias) @ rel_k_b
which turns a 21-GFLOP tensor contraction into a single streaming pass over
rel_kernel (655 MB) in bf16 on the Vector engine: elementwise multiply by u
(free-dim-broadcast AP) + pairwise tree reduction over d.

DVE op cost is free-dim-size bound, so every streaming chunk must use all
128 partitions. Per rel head hr, the 2x200 (batch, i) rows are packed as:
  blocks A/B/C: partition = (b in 2) x (i-halfblock of 64) -> 128 rows
  block D (i in [192,200)): partition = (jq in 8) x (b in 2) x (i in 8),
    with j split in 8 chunks of 25; uses a jq-replicated u operand, and its
    s2 result is unpacked back to row-major with 8 identity-slice matmuls.
All PE writes land on 0/64 partition bases (hardware requires 32-aligned).

Softmax skips the max-subtraction (logits are O(1); masked entries are -1e9
and exp underflows to exactly 0); p normalized on the Scalar engine in bf16.

Sharding: data-parallel over batch, B=16 -> 2 batches per core on 8 cores.
"""

import numpy as np
from contextlib import ExitStack

import concourse.bass as bass
import concourse.bacc as bacc
import concourse.tile as tile
from concourse import mybir
from concourse.masks import make_identity
from concourse.bass_utils import run_bass_kernel_spmd

B, T, D = 16, 200, 128
H_ABS, H_REL, H, DK = 2, 2, 4, 32
N_CORES = 8
BPC = B // N_CORES  # batches per core
SCALE = 1.0 / float(DK) ** 0.5
JC = 50  # j-chunk for full streaming blocks (T % JC == 0)
TT = BPC * T  # tokens per core (400)

F32 = mybir.dt.float32
BF16 = mybir.dt.bfloat16
I32 = mybir.dt.int32
AX = mybir.AxisListType
OP = mybir.AluOpType
AF = mybir.ActivationFunctionType

# abs-head i-blocks per batch: (start, len)
IBLOCKS = [(0, 128), (128, T - 128)]
# full stream blocks per hr: i-halfblocks of 64, two batches stacked
HB = 64
FULL_I0 = [0, 64, 128]
DN = T - 192  # 8 leftover i-rows -> packed block D
DJ = T // 8   # 25: j-chunk for block D


def build_kernel(ctx: ExitStack, tc: tile.TileContext, io: dict):
    nc = tc.nc

    query = io["query"].flatten_outer_dims()  # [400, 128]
    key = io["key"].flatten_outer_dims()
    value = io["value"].flatten_outer_dims()
    mask = io["mask"]          # [2, 1, 200, 200] i32
    relk = io["rel_kernel"]    # [2, 2, 200, 200, 128] bf16
    absk = io["abs_kernel"]    # [2, 2, 200, 128]
    out = io["out"]            # [2, 200, 128]

    consts = ctx.enter_context(tc.tile_pool(name="consts", bufs=1))
    prep = ctx.enter_context(tc.tile_pool(name="prep", bufs=2))
    keep = ctx.enter_context(tc.tile_pool(name="keep", bufs=1))

    def dma_sync(out_ap, in_ap):
        return nc.sync.dma_start(out_ap, in_ap)

    def dma_scalar(out_ap, in_ap):
        return nc.scalar.dma_start(out_ap, in_ap)

    ident = consts.tile([128, 128], F32, tag="ident")
    make_identity(nc, ident)
    identb = consts.tile([128, 128], BF16, tag="identb")
    nc.gpsimd.tensor_copy(identb, ident)

    def load_const(name, ap, shape, eng=dma_scalar):
        t = consts.tile(shape, F32, tag=name)
        eng(t, ap)
        return t

    with tc.tile_pool(name="psum_prep", bufs=2, space="PSUM") as psp, \
         tc.tile_pool(name="psum_prep1", bufs=1, space="PSUM") as psp1:

        # transpose token-major inputs to [din, t]
        def transpose_in(src_ap, tag, eng):
            xt = keep.tile([128, TT], F32, tag=tag)
            for ti, t0 in enumerate(range(0, TT, 128)):
                tl = min(128, TT - t0)
                raw = prep.tile([128, 128], F32, tag="t_raw")
                eng(raw[:tl, :], src_ap[t0 : t0 + tl, :])
                tp = psp.tile([128, 128], F32, tag="t_ps")
                nc.tensor.transpose(tp[:, :tl], raw[:tl, :], ident[:tl, :tl])
                nc.scalar.copy(xt[:, t0 : t0 + tl], tp[:, :tl])
            return xt

        # ---- critical path first: everything the streaming pass needs ----
        xqT = transpose_in(query, "xqT", dma_sync)

        wq = load_const("wq", io["Wq"], [128, 128], dma_sync)
        bq_c = load_const("bq", io["bq"], [128, 1], dma_sync)
        bq_s = consts.tile([128, 1], F32, tag="bq_s")
        nc.scalar.activation(bq_s, bq_c, AF.Copy, scale=SCALE)

        rkw = {}
        small_cols = {}
        for hr in range(H_REL):
            rkw[hr] = load_const(f"rkw{hr}", io["rel_k_w"][hr], [128, DK],
                                 dma_sync)
            small_cols[("rkb", hr)] = load_const(
                f"rkb{hr}", io["rel_k_b"][hr], [DK, 1], dma_sync)
            t = load_const(f"rbias{hr}", io["rel_bias"][0, hr, 0, :], [DK, 1],
                           dma_sync)
            ts_ = consts.tile([DK, 1], F32, tag=f"rbias_s{hr}")
            nc.scalar.activation(ts_, t, AF.Copy, scale=SCALE)
            small_cols[("rbias_s", hr)] = ts_

        rkwT = {}
        for hr in range(H_REL):
            tp = psp.tile([DK, 128], F32, tag="mm_ps")
            nc.tensor.transpose(tp, rkw[hr], ident)
            t = keep.tile([DK, 128], F32, tag=f"rkwT{hr}")
            nc.scalar.copy(t, tp)
            rkwT[hr] = t

        qT = {}
        for h in (H_ABS, H_ABS + 1, 0, 1):  # rel heads first
            qp = psp.tile([DK, TT], F32, tag="mm_ps")
            nc.tensor.matmul(qp, wq[:, DK * h : DK * (h + 1)], xqT)
            t = keep.tile([DK, TT], F32, tag=f"qT{h}")
            nc.scalar.activation(t, qp, AF.Identity,
                                 bias=bq_s[DK * h : DK * (h + 1)], scale=SCALE)
            qT[h] = t

        qrbT = {}
        for hr in range(H_REL):
            t = keep.tile([DK, TT], F32, tag=f"qrbT{hr}")
            nc.vector.tensor_scalar(t, qT[H_ABS + hr],
                                    small_cols[("rbias_s", hr)], None, OP.add)
            qrbT[hr] = t

        # u/c for full blocks: partition = b*64 + (i - i0)
        u_blk = {}
        c_blk = {}
        for hr in range(H_REL):
            for i0 in FULL_I0:
                up = psp1.tile([128, 128], F32, tag="sm_ps")
                cp = psp1.tile([128, 1], F32, tag="sm_psc")
                for b in range(BPC):
                    tsl = slice(b * T + i0, b * T + i0 + HB)
                    nc.tensor.matmul(up[b * HB : (b + 1) * HB, :],
                                     qrbT[hr][:, tsl], rkwT[hr])
                    nc.tensor.matmul(cp[b * HB : (b + 1) * HB, :],
                                     qrbT[hr][:, tsl],
                                     small_cols[("rkb", hr)])
                t = keep.tile([128, 128], BF16, tag=f"ub{hr}_{i0}")
                nc.scalar.copy(t, up)
                u_blk[(hr, i0)] = t
                t = keep.tile([128, 1], F32, tag=f"cb{hr}_{i0}")
                nc.scalar.copy(t, cp)
                c_blk[(hr, i0)] = t

        # u/c for block D (i in [192, 200), both b): 16 rows, row = b*8+(i-192)
        # b1 window first covering [0:16) (8 garbage rows), then b0 [0:8).
        # packed partition p = b*64 + r*8 + jq -> REP = ident16 (x) ones8
        u16 = {}
        c16 = {}
        rep16 = consts.tile([16, 128], BF16, tag="rep16")
        nc.vector.memset(rep16, 0.0)
        rep_view = bass.AP(tensor=rep16.tensor, offset=rep16.offset,
                           ap=[[128, 16], [8, 16], [1, 8]])
        nc.vector.tensor_copy(
            rep_view,
            identb[:16, :16].unsqueeze(2).broadcast_to([16, 16, 8]))
        for hr in range(H_REL):
            up = psp1.tile([16, 128], F32, tag="sm_ps16")
            cp = psp1.tile([16, 1], F32, tag="sm_psc16")
            nc.tensor.matmul(up[0:16, :], qrbT[hr][:, TT - 16 : TT], rkwT[hr])
            nc.tensor.matmul(up[0:8, :], qrbT[hr][:, 192:200], rkwT[hr])
            nc.tensor.matmul(cp[0:16, :], qrbT[hr][:, TT - 16 : TT],
                             small_cols[("rkb", hr)])
            nc.tensor.matmul(cp[0:8, :], qrbT[hr][:, 192:200],
                             small_cols[("rkb", hr)])
            t16 = keep.tile([16, 128], BF16, tag=f"u16_{hr}")
            nc.scalar.copy(t16, up)
            u16[hr] = t16
            tc16 = keep.tile([16, 1], F32, tag=f"c16_{hr}")
            nc.scalar.copy(tc16, cp)
            c16[hr] = tc16
        # replicate u16 8x along partitions: u_rep[jq*16 + r] = u16[r]
        u_rep = {}
        for hr in range(H_REL):
            urp = psp1.tile([128, 128], F32, tag="sm_ps")
            nc.tensor.matmul(urp, rep16, u16[hr])
            t = keep.tile([128, 128], BF16, tag=f"urep{hr}")
            nc.scalar.copy(t, urp)
            u_rep[hr] = t

        # ---- rest of prep (scalar ring) ----
        xkT = transpose_in(key, "xkT", dma_scalar)
        xvT = transpose_in(value, "xvT", dma_scalar)

        wk = load_const("wk", io["Wk"], [128, 128])
        wv = load_const("wv", io["Wv"], [128, 128])
        wo = load_const("wo", io["Wo"], [128, 128])
        bk_c = load_const("bk", io["bk"], [128, 1])
        bv_b = consts.tile([128, 128], F32, tag="bv_b")
        bv_ap = io["bv"]
        dma_scalar(bv_b, bass.AP(tensor=bv_ap.tensor, offset=bv_ap.offset,
                                 ap=[[0, 128]] + bv_ap.ap))
        bo_b = consts.tile([128, 128], F32, tag="bo_b")
        bo_ap = io["bo"]
        dma_scalar(bo_b, bass.AP(tensor=bo_ap.tensor, offset=bo_ap.offset,
                                 ap=[[0, 128]] + bo_ap.ap))

        abs_w = {}
        for hh in range(H_ABS):
            abs_w[("aqw", hh)] = load_const(f"aqw{hh}", io["abs_q_w"][hh], [128, DK])
            abs_w[("akw", hh)] = load_const(f"akw{hh}", io["abs_k_w"][hh], [128, DK])
            small_cols[("akb", hh)] = load_const(
                f"akb{hh}", io["abs_k_b"][hh], [DK, 1])
            t = load_const(f"aqb{hh}", io["abs_q_b"][hh], [DK, 1])
            ts_ = consts.tile([DK, 1], F32, tag=f"aqb_s{hh}")
            nc.scalar.activation(ts_, t, AF.Copy, scale=SCALE)
            small_cols[("aqb_s", hh)] = ts_

        kT = {}
        for h in range(H):
            kp = psp.tile([DK, TT], F32, tag="mm_ps")
            nc.tensor.matmul(kp, wk[:, DK * h : DK * (h + 1)], xkT)
            t = keep.tile([DK, TT], F32, tag=f"kT{h}")
            nc.scalar.activation(t, kp, AF.Identity,
                                 bias=bk_c[DK * h : DK * (h + 1)])
            kT[h] = t

        # v without bias: softmax rows sum to 1, so bv folds into the output
        # bias: y = (p@v0) @ Wo + (bv @ Wo + bo)
        vb = {}
        for b in range(BPC):
            for jb, (j0, jl) in enumerate(IBLOCKS):
                vp = psp1.tile([128, 128], F32, tag="sm_ps")
                nc.tensor.matmul(vp[:jl, :], xvT[:, b * T + j0 : b * T + j0 + jl], wv)
                t = keep.tile([128, 128], BF16, tag=f"v{b}_{jb}")
                nc.scalar.copy(t[:jl, :], vp[:jl, :])
                vb[(b, jb)] = t
        # bo' = bv @ Wo + bo, broadcast over partitions via a PE ones-matmul
        bop = psp1.tile([1, 128], F32, tag="sm_ps16")
        nc.tensor.matmul(bop, bv_b[:, 0:1], wo)
        bo_r = keep.tile([1, 128], F32, tag="bo_r")
        nc.vector.tensor_add(bo_r, bop, bo_b[0:1, :])
        ones1 = consts.tile([1, 128], F32, tag="ones1")
        nc.vector.memset(ones1, 1.0)
        bobp = psp1.tile([128, 128], F32, tag="sm_ps")
        nc.tensor.matmul(bobp, ones1, bo_r)
        bo_bb = keep.tile([128, 128], F32, tag="bo_bb")
        nc.scalar.copy(bo_bb, bobp)

        qaT = {}
        kaT = {}
        for hh in range(H_ABS):
            akT = transpose_in(absk[hh].flatten_outer_dims(), f"akT{hh}",
                               dma_scalar)
            pp = psp.tile([DK, TT], F32, tag="mm_ps")
            nc.tensor.matmul(pp, abs_w[("aqw", hh)], akT)
            pqT = prep.tile([DK, TT], F32, tag="pqT")
            nc.scalar.activation(pqT, pp, AF.Identity,
                                 bias=small_cols[("aqb_s", hh)], scale=SCALE)
            t = keep.tile([DK, TT], F32, tag=f"qaT{hh}")
            nc.vector.tensor_add(t, qT[hh], pqT)
            qaT[hh] = t

            pp2 = psp.tile([DK, TT], F32, tag="mm_ps")
            nc.tensor.matmul(pp2, abs_w[("akw", hh)], akT)
            pkT = prep.tile([DK, TT], F32, tag="pqT")
            nc.scalar.activation(pkT, pp2, AF.Identity,
                                 bias=small_cols[("akb", hh)])
            t = keep.tile([DK, TT], F32, tag=f"kaT{hh}")
            nc.vector.tensor_add(t, kT[hh], pkT)
            kaT[hh] = t

        # mask tiles: (b, ib) blocks for abs heads
        mb_abs = {}
        for b in range(BPC):
            for ib, (i0, il) in enumerate(IBLOCKS):
                mi = prep.tile([128, T], I32, tag="m_i32")
                dma_scalar(mi[:il, :], mask[b, 0, i0 : i0 + il, :])
                t = keep.tile([128, T], F32, tag=f"mb{b}_{ib}")
                nc.vector.tensor_scalar(t[:il, :], mi[:il, :], 1e9, -1e9,
                                        OP.mult, OP.add)
                mb_abs[(b, ib)] = t

        # mask tiles for stream blocks: partition = b*64 + (i - i0)
        mb_blk = {}
        for i0 in FULL_I0:
            mi = prep.tile([128, T], I32, tag="ms_i32")
            for b in range(BPC):
                dma_scalar(mi[b * HB : (b + 1) * HB, :],
                           mask[b, 0, i0 : i0 + HB, :])
            t = keep.tile([128, T], F32, tag=f"mbs{i0}")
            nc.vector.tensor_scalar(t, mi, 1e9, -1e9, OP.mult, OP.add)
            mb_blk[i0] = t
        mi = prep.tile([16, T], I32, tag="ms_i32l")
        for b in range(BPC):
            dma_scalar(mi[b * DN : (b + 1) * DN, :], mask[b, 0, 192:T, :])
        mb16 = keep.tile([16, T], F32, tag="mb16")
        nc.vector.tensor_scalar(mb16, mi, 1e9, -1e9, OP.mult, OP.add)

    # ---------------- main phase ----------------
    stream = ctx.enter_context(tc.tile_pool(name="stream", bufs=6))
    wpool = ctx.enter_context(tc.tile_pool(name="wpool", bufs=2))
    tree = ctx.enter_context(tc.tile_pool(name="tree", bufs=2))
    s2pool = ctx.enter_context(tc.tile_pool(name="s2pool", bufs=1))
    sm = ctx.enter_context(tc.tile_pool(name="sm", bufs=2))
    ps_s1 = ctx.enter_context(tc.tile_pool(name="ps_s1", bufs=2, space="PSUM"))
    ps_tp = ctx.enter_context(tc.tile_pool(name="ps_tp", bufs=2, space="PSUM"))
    ps_x = ctx.enter_context(tc.tile_pool(name="ps_x", bufs=1, space="PSUM"))
    ps_tail = ctx.enter_context(tc.tile_pool(name="ps_tail", bufs=1, space="PSUM"))

    chunk_n = [0]

    def stream_chunk(dram_aps, s2t, ub, jslice, jcw):
        """One [128, jcw, 128] chunk: dma halves, mult by ub, d-tree-reduce.
        dram_aps: list of (partition_slice, src_ap) — each src must have a
        single leading partition dim so the DMA spreads across engines."""
        rk = stream.tile([128, JC, 128], BF16, tag="rk")
        chunk_n[0] += 1
        dma_eng = nc.sync if chunk_n[0] % 2 == 0 else nc.scalar
        for psl, src in dram_aps:
            dma_eng.dma_start(rk[psl, :jcw, :], src)
        w = wpool.tile([128, JC, 128], BF16, tag="w")
        nc.vector.tensor_tensor(
            w[:, :jcw, :], rk[:, :jcw, :],
            ub.unsqueeze(1).broadcast_to([128, jcw, 128]), op=OP.mult)
        cur = w
        width = 64
        while width >= 2:
            nxt = tree.tile([128, JC, width], BF16, tag=f"L{width}")
            nc.vector.tensor_add(nxt[:, :jcw, :], cur[:, :jcw, 0:width],
                                 cur[:, :jcw, width : 2 * width])
            cur = nxt
            width //= 2
        nc.vector.tensor_add(s2t[:, jslice], cur[:, :jcw, 0],
                             cur[:, :jcw, 1])

    # PSUM x accumulator: one bank, columns (b*2+ib)*128 + h*DK per head
    x_all = ps_x.tile([128, 512], F32, tag="x_all", name="x_all")

    def x_col(b, ib, h):
        return (b * 2 + ib) * 128 + DK * h

    def softmax_pv(st, rows, segs, h_of_seg):
        """exp + rowsum + normalize + transpose/pv for one score block.
        st: [rows, T] logits (SBUF). segs: [(off, ln, b, i0)]."""
        p = sm.tile([128, T], BF16, tag="p")
        rsum = sm.tile([128, 1], F32, tag="rsum")
        nc.scalar.activation(p[:rows, :], st[:rows, :], AF.Exp,
                             accum_out=rsum[:rows])
        rcp = sm.tile([128, 1], F32, tag="rcp")
        nc.vector.reciprocal(rcp[:rows], rsum[:rows])
        pn = sm.tile([128, T], BF16, tag="pn")
        nc.scalar.activation(pn[:rows, :], p[:rows, :], AF.Copy,
                             scale=rcp[:rows])
        for (off, ln, b, i0), h in zip(segs, h_of_seg):
            ib = 0 if i0 < 128 else 1
            xoff = i0 - ib * 128
            xc = x_col(b, ib, h)
            for jb, (j0, jl) in enumerate(IBLOCKS):
                al = (off // 64) * 64  # 64-aligned covering slice for PE read
                ln_c = off + ln - al
                tp = ps_tp.tile([128, 128], BF16, tag="tp")
                nc.tensor.transpose(tp[:jl, :ln_c],
                                    pn[al : off + ln, j0 : j0 + jl],
                                    identb[al : off + ln, al : off + ln])
                pT = sm.tile([128, 128], BF16, tag="pT")
                nc.scalar.copy(pT[:jl, :ln_c], tp[:jl, :ln_c])
                nc.tensor.matmul(x_all[xoff : xoff + ln, xc : xc + DK],
                                 pT[:jl, off - al : off - al + ln],
                                 vb[(b, jb)][:jl, DK * h : DK * (h + 1)],
                                 start=(jb == 0), stop=(jb == 1))

    # ---- abs-head scores: emitted interleaved between stream blocks so the
    # DVE queue head stays on stream work at startup ----
    abs_jobs = [(b, ib, h) for b in range(BPC) for ib in range(2)
                for h in range(H_ABS)]

    def emit_abs(n):
        for _ in range(n):
            if not abs_jobs:
                return
            b, ib, h = abs_jobs.pop(0)
            i0, il = IBLOCKS[ib]
            tsl = slice(b * T + i0, b * T + i0 + il)
            s1 = ps_s1.tile([128, T], F32, tag="s1")
            nc.tensor.matmul(s1[:il, :], qaT[h][:, tsl],
                             kaT[h][:, b * T : (b + 1) * T])
            st = sm.tile([128, T], F32, tag="st")
            nc.vector.tensor_add(st[:il, :], s1[:il, :],
                                 mb_abs[(b, ib)][:il, :])
            softmax_pv(st, il, [(0, il, b, i0)], [h])

    # ---- the stream + rel scores ----
    def rel_scores_full(hr, i0, s2t):
        h = H_ABS + hr
        s1 = ps_s1.tile([128, T], F32, tag="s1")
        for b in range(BPC):
            nc.tensor.matmul(s1[b * HB : (b + 1) * HB, :],
                             qT[h][:, b * T + i0 : b * T + i0 + HB],
                             kT[h][:, b * T : (b + 1) * T])
        st = sm.tile([128, T], F32, tag="st")
        nc.vector.scalar_tensor_tensor(st, s1, c_blk[(hr, i0)], s2t,
                                       op0=OP.add, op1=OP.add)
        nc.vector.tensor_add(st, st, mb_blk[i0])
        softmax_pv(st, 128,
                   [(b * HB, HB, b, i0) for b in range(BPC)], [h, h])

    rel_stride = T * D  # row stride in rel_kernel elements
    for hr in range(H_REL):
        base = relk[hr]  # [2, 200, 200, 128] -> b, i, j, d
        for i0 in FULL_I0:
            s2t = s2pool.tile([128, T], F32, tag=f"s2_{hr}_{i0}",
                              name=f"s2_{hr}_{i0}")
            for jc0 in range(0, T, JC):
                # partition = (b in 2, i-i0 in 64); free = (j in 50, d)
                aps = []
                for b in range(BPC):
                    src = bass.AP(
                        tensor=base.tensor,
                        offset=(base.offset + (b * T + i0) * rel_stride
                                + jc0 * D),
                        ap=[[rel_stride, HB], [D, JC], [1, D]])
                    aps.append((slice(b * HB, (b + 1) * HB), src))
                stream_chunk(aps, s2t, u_blk[(hr, i0)],
                             slice(jc0, jc0 + JC), JC)
            rel_scores_full(hr, i0, s2t)
            emit_abs(1 if (hr, i0) != (0, 0) else 2)

        # block D: partition p = b*64 + r*8 + jq; free = (j' 25, d).
        # r and jq strides merge: r-stride (T*D) = 8 * jq-stride (DJ*D).
        s2p = s2pool.tile([128, DJ], F32, tag=f"s2p_{hr}", name=f"s2p_{hr}")
        aps = []
        for b in range(BPC):
            src = bass.AP(
                tensor=base.tensor,
                offset=base.offset + (b * T + 192) * rel_stride,
                ap=[[DJ * D, DN * 8], [1, DJ * D]])
            aps.append((slice(b * HB, (b + 1) * HB), src))
        stream_chunk(aps, s2p, u_rep[hr], slice(0, DJ), DJ)
        # unpack: s2_16[row, jq*25+j'] = s2p[8*row + jq, j']
        # lhsT for jq = ident columns [jq::8][:16] (stride-8 column view)
        s2l_ps = ps_tail.tile([16, T], F32, tag="s2l")
        for jq in range(8):
            e_jq = bass.AP(tensor=ident.tensor, offset=ident.offset + jq,
                           ap=[[128, 128], [8, 16]])
            nc.tensor.matmul(s2l_ps[:, jq * DJ : (jq + 1) * DJ],
                             e_jq, s2p[:, 0:DJ], start=True, stop=True)
        s2_16 = sm.tile([16, T], F32, tag="s2_16")
        nc.scalar.copy(s2_16, s2l_ps)

        # block D scores: 16 rows, row = b*8 + (i-192)
        h = H_ABS + hr
        s1 = ps_s1.tile([128, T], F32, tag="s1")
        nc.tensor.matmul(s1[0:16, :], qT[h][:, TT - 16 : TT],
                         kT[h][:, T : 2 * T])
        nc.tensor.matmul(s1[0:8, :], qT[h][:, 192:200], kT[h][:, 0:T])
        st = sm.tile([16, T], F32, tag="std")
        nc.vector.scalar_tensor_tensor(st, s1[0:16, :], c16[hr], s2_16,
                                       op0=OP.add, op1=OP.add)
        nc.vector.tensor_add(st, st, mb16)
        softmax_pv(st, 16, [(b * DN, DN, b, 192) for b in range(BPC)],
                   [h, h])
        emit_abs(1)

    emit_abs(8)

    # ---- output: x scaling already folded; project ----
    for b in range(BPC):
        for ib, (i0, il) in enumerate(IBLOCKS):
            xc = (b * 2 + ib) * 128
            x_sb = sm.tile([128, 128], F32, tag="x_sb")
            nc.scalar.copy(x_sb[:il, :], x_all[:il, xc : xc + 128])
            xT_ps = ps_tail.tile([128, 128], F32, tag="tail3")
            nc.tensor.transpose(xT_ps[:, :il], x_sb[:il, :], ident[:il, :il])
            xT_sb = sm.tile([128, 128], F32, tag="xT_sb")
            nc.scalar.copy(xT_sb[:, :il], xT_ps[:, :il])
            y_ps = ps_tail.tile([128, 128], F32, tag="tail3")
            nc.tensor.matmul(y_ps[:il, :], xT_sb[:, :il], wo)
            y_sb = sm.tile([128, 128], F32, tag="y_sb")
            nc.vector.tensor_add(y_sb[:il, :], y_ps[:il, :], bo_bb[:il, :])
            nc.sync.dma_start(out[b, i0 : i0 + il, :], y_sb[:il, :])


def build_nc():
    nc = bacc.Bacc(trn_type="TRN2")
    io = {}
    io["query"] = nc.dram_tensor("query", [BPC, T, D], F32, kind="ExternalInput").ap()
    io["key"] = nc.dram_tensor("key", [BPC, T, D], F32, kind="ExternalInput").ap()
    io["value"] = nc.dram_tensor("value", [BPC, T, D], F32, kind="ExternalInput").ap()
    io["mask"] = nc.dram_tensor("mask", [BPC, 1, T, T], I32, kind="ExternalInput").ap()
    io["rel_kernel"] = nc.dram_tensor(
        "rel_kernel", [H_REL, BPC, T, T, D], BF16, kind="ExternalInput"
    ).ap()
    io["abs_kernel"] = nc.dram_tensor(
        "abs_kernel", [H_ABS, BPC, T, D], F32, kind="ExternalInput"
    ).ap()
    for nm, shape in [
        ("Wq", [D, D]), ("bq", [D]), ("Wk", [D, D]), ("bk", [D]),
        ("Wv", [D, D]), ("bv", [D]),
        ("abs_q_w", [H_ABS, D, DK]), ("abs_q_b", [H_ABS, DK]),
        ("abs_k_w", [H_ABS, D, DK]), ("abs_k_b", [H_ABS, DK]),
        ("rel_k_w", [H_REL, D, DK]), ("rel_k_b", [H_REL, DK]),
        ("rel_bias", [1, H_REL, 1, DK]),
        ("Wo", [D, D]), ("bo", [D]),
    ]:
        io[nm] = nc.dram_tensor(nm, shape, F32, kind="ExternalInput").ap()
    io["out"] = nc.dram_tensor("out", [BPC, T, D], F32, kind="ExternalOutput").ap()

    with tile.TileContext(nc) as tc:
        with ExitStack() as ctx:
            build_kernel(ctx, tc, io)
    nc.compile()
    return nc


_NC_CACHE = None


def _get_nc():
    global _NC_CACHE
    if _NC_CACHE is None:
        _NC_CACHE = build_nc()
    return _NC_CACHE


def make_in_maps(inputs):
    """Shard full inputs into per-core input maps."""
    f32 = np.float32
    weights = {
        nm: np.ascontiguousarray(np.asarray(inputs[nm], dtype=f32))
        for nm in ["Wq", "bq", "Wk", "bk", "Wv", "bv", "abs_q_w", "abs_q_b",
                   "abs_k_w", "abs_k_b", "rel_k_w", "rel_k_b", "rel_bias",
                   "Wo", "bo"]
    }
    query = np.asarray(inputs["query"], dtype=f32)
    key = np.asarray(inputs["key"], dtype=f32)
    value = np.asarray(inputs["value"], dtype=f32)
    mask = np.asarray(inputs["mask"], dtype=np.int32)
    import ml_dtypes
    relk = np.asarray(inputs["rel_kernel"], dtype=f32).astype(ml_dtypes.bfloat16)
    absk = np.asarray(inputs["abs_kernel"], dtype=f32)

    in_maps = []
    for c in range(N_CORES):
        bs = slice(c * BPC, (c + 1) * BPC)
        m = dict(weights)
        m["query"] = np.ascontiguousarray(query[bs])
        m["key"] = np.ascontiguousarray(key[bs])
        m["value"] = np.ascontiguousarray(value[bs])
        m["mask"] = np.ascontiguousarray(mask[bs])
        m["rel_kernel"] = np.ascontiguousarray(relk[:, bs])
        m["abs_kernel"] = np.ascontiguousarray(absk[:, bs])
        in_maps.append(m)
    return in_maps


def kernel(**inputs) -> np.ndarray:
    nc = _get_nc()
    in_maps = make_in_maps(inputs)
    res = run_bass_kernel_spmd(nc, in_maps, core_ids=list(range(N_CORES)))
    return np.concatenate([r["out"] for r in res.results], axis=0)


if __name__ == "__main__":
    nc = build_nc()
    print("built ok")


# revision 40
# speedup vs baseline: 3.0907x; 1.0294x over previous
"""Trainium2 Bass kernel for nn_Attention_12463995093474 (sparse_attention).

Math (reference):
  q/k/v = content linears; 2 absolute heads, 2 relative heads (DK=32).
  abs:  scores = (Xq_a + abs_kernel@abs_q_w) @ (Xk_a + abs_kernel@abs_k_w)^T
  rel:  scores = Xq_r @ Xk_r^T + (Xq_r + rel_bias) . (rel_kernel@rel_k_w + rel_k_b)
  softmax(mask) @ v -> out linear.

Key optimization: the dominant term
    sum_o (Xq_r+rel_bias)[i,o] * (sum_d rel_kernel[i,j,d] rel_k_w[d,o] + rel_k_b[o])
is reassociated to
    sum_d rel_kernel[i,j,d] * u[i,d] + c[i],
      u = rel_k_w @ (Xq_r+rel_bias)^T,  c = (Xq_r+rel_bias) @ rel_k_b
which turns a 21-GFLOP tensor contraction into a single streaming pass over
rel_kernel (655 MB) in bf16 on the Vector engine: elementwise multiply by u
(free-dim-broadcast AP) + pairwise tree reduction over d.

DVE op cost is free-dim-size bound, so every streaming chunk must use all
128 partitions. Per rel head hr, the 2x200 (batch, i) rows are packed as:
  blocks A/B/C: partition = (b in 2) x (i-halfblock of 64) -> 128 rows
  block D (i in [192,200)): partition = (jq in 8) x (b in 2) x (i in 8),
    with j split in 8 chunks of 25; uses a jq-replicated u operand, and its
    s2 result is unpacked back to row-major with 8 identity-slice matmuls.
All PE writes land on 0/64 partition bases (hardware requires 32-aligned).

Softmax skips the max-subtraction (logits are O(1); masked entries are -1e9
and exp underflows to exactly 0); p normalized on the Scalar engine in bf16.

Sharding: data-parallel over batch, B=16 -> 2 batches per core on 8 cores.
"""

import numpy as np
from contextlib import ExitStack

import concourse.bass as bass
import concourse.bacc as bacc
import concourse.tile as tile
from concourse import mybir
from concourse.masks import make_identity
from concourse.bass_utils import run_bass_kernel_spmd

B, T, D = 16, 200, 128
H_ABS, H_REL, H, DK = 2, 2, 4, 32
N_CORES = 8
BPC = B // N_CORES  # batches per core
SCALE = 1.0 / float(DK) ** 0.5
JC = 50  # j-chunk for full streaming blocks (T % JC == 0)
TT = BPC * T  # tokens per core (400)

F32 = mybir.dt.float32
BF16 = mybir.dt.bfloat16
I32 = mybir.dt.int32
AX = mybir.AxisListType
OP = mybir.AluOpType
AF = mybir.ActivationFunctionType

# abs-head i-blocks per batch: (start, len)
IBLOCKS = [(0, 128), (128, T - 128)]
# full stream blocks per hr: i-halfblocks of 64, two batches stacked
HB = 64
FULL_I0 = [0, 64, 128]
DN = T - 192  # 8 leftover i-rows -> packed block D
DJ = T // 8   # 25: j-chunk for block D


def build_kernel(ctx: ExitStack, tc: tile.TileContext, io: dict):
    nc = tc.nc

    query = io["query"].flatten_outer_dims()  # [400, 128]
    key = io["key"].flatten_outer_dims()
    value = io["value"].flatten_outer_dims()
    mask = io["mask"]          # [2, 1, 200, 200] i32
    relk = io["rel_kernel"]    # [2, 2, 200, 200, 128] bf16
    absk = io["abs_kernel"]    # [2, 2, 200, 128]
    out = io["out"]            # [2, 200, 128]

    consts = ctx.enter_context(tc.tile_pool(name="consts", bufs=1))
    prep = ctx.enter_context(tc.tile_pool(name="prep", bufs=2))
    keep = ctx.enter_context(tc.tile_pool(name="keep", bufs=1))

    def dma_sync(out_ap, in_ap):
        return nc.sync.dma_start(out_ap, in_ap)

    def dma_scalar(out_ap, in_ap):
        return nc.scalar.dma_start(out_ap, in_ap)

    ident = consts.tile([128, 128], F32, tag="ident")
    make_identity(nc, ident)
    identb = consts.tile([128, 128], BF16, tag="identb")
    nc.gpsimd.tensor_copy(identb, ident)

    def load_const(name, ap, shape, eng=dma_scalar):
        t = consts.tile(shape, F32, tag=name)
        eng(t, ap)
        return t

    with tc.tile_pool(name="psum_prep", bufs=2, space="PSUM") as psp, \
         tc.tile_pool(name="psum_prep1", bufs=1, space="PSUM") as psp1:

        # transpose token-major inputs to [din, t]
        def transpose_in(src_ap, tag, eng):
            xt = keep.tile([128, TT], F32, tag=tag)
            for ti, t0 in enumerate(range(0, TT, 128)):
                tl = min(128, TT - t0)
                raw = prep.tile([128, 128], F32, tag="t_raw")
                eng(raw[:tl, :], src_ap[t0 : t0 + tl, :])
                tp = psp.tile([128, 128], F32, tag="t_ps")
                nc.tensor.transpose(tp[:, :tl], raw[:tl, :], ident[:tl, :tl])
                nc.scalar.copy(xt[:, t0 : t0 + tl], tp[:, :tl])
            return xt

        # ---- critical path first: everything the streaming pass needs ----
        xqT = transpose_in(query, "xqT", dma_sync)

        wq = load_const("wq", io["Wq"], [128, 128], dma_sync)
        bq_c = load_const("bq", io["bq"], [128, 1], dma_sync)
        bq_s = consts.tile([128, 1], F32, tag="bq_s")
        nc.scalar.activation(bq_s, bq_c, AF.Copy, scale=SCALE)

        rkw = {}
        small_cols = {}
        for hr in range(H_REL):
            rkw[hr] = load_const(f"rkw{hr}", io["rel_k_w"][hr], [128, DK],
                                 dma_sync)
            small_cols[("rkb", hr)] = load_const(
                f"rkb{hr}", io["rel_k_b"][hr], [DK, 1], dma_sync)
            t = load_const(f"rbias{hr}", io["rel_bias"][0, hr, 0, :], [DK, 1],
                           dma_sync)
            ts_ = consts.tile([DK, 1], F32, tag=f"rbias_s{hr}")
            nc.scalar.activation(ts_, t, AF.Copy, scale=SCALE)
            small_cols[("rbias_s", hr)] = ts_

        rkwT = {}
        for hr in range(H_REL):
            tp = psp.tile([DK, 128], F32, tag="mm_ps")
            nc.tensor.transpose(tp, rkw[hr], ident)
            t = keep.tile([DK, 128], F32, tag=f"rkwT{hr}")
            nc.scalar.copy(t, tp)
            rkwT[hr] = t

        qT = {}
        for h in (H_ABS, H_ABS + 1, 0, 1):  # rel heads first
            qp = psp.tile([DK, TT], F32, tag="mm_ps")
            nc.tensor.matmul(qp, wq[:, DK * h : DK * (h + 1)], xqT)
            t = keep.tile([DK, TT], F32, tag=f"qT{h}")
            nc.scalar.activation(t, qp, AF.Identity,
                                 bias=bq_s[DK * h : DK * (h + 1)], scale=SCALE)
            qT[h] = t

        qrbT = {}
        for hr in range(H_REL):
            t = keep.tile([DK, TT], F32, tag=f"qrbT{hr}")
            nc.vector.tensor_scalar(t, qT[H_ABS + hr],
                                    small_cols[("rbias_s", hr)], None, OP.add)
            qrbT[hr] = t

        # u/c for full blocks: partition = b*64 + (i - i0)
        u_blk = {}
        c_blk = {}
        for hr in range(H_REL):
            for i0 in FULL_I0:
                up = psp1.tile([128, 128], F32, tag="sm_ps")
                cp = psp1.tile([128, 1], F32, tag="sm_psc")
                for b in range(BPC):
                    tsl = slice(b * T + i0, b * T + i0 + HB)
                    nc.tensor.matmul(up[b * HB : (b + 1) * HB, :],
                                     qrbT[hr][:, tsl], rkwT[hr])
                    nc.tensor.matmul(cp[b * HB : (b + 1) * HB, :],
                                     qrbT[hr][:, tsl],
                                     small_cols[("rkb", hr)])
                t = keep.tile([128, 128], BF16, tag=f"ub{hr}_{i0}")
                nc.scalar.copy(t, up)
                u_blk[(hr, i0)] = t
                t = keep.tile([128, 1], F32, tag=f"cb{hr}_{i0}")
                nc.scalar.copy(t, cp)
                c_blk[(hr, i0)] = t

        # u/c for block D (i in [192, 200), both b): 16 rows, row = b*8+(i-192)
        # b1 window first covering [0:16) (8 garbage rows), then b0 [0:8).
        # packed partition p = b*64 + r*8 + jq -> REP = ident16 (x) ones8
        u16 = {}
        c16 = {}
        rep16 = consts.tile([16, 128], BF16, tag="rep16")
        nc.vector.memset(rep16, 0.0)
        rep_view = bass.AP(tensor=rep16.tensor, offset=rep16.offset,
                           ap=[[128, 16], [8, 16], [1, 8]])
        nc.vector.tensor_copy(
            rep_view,
            identb[:16, :16].unsqueeze(2).broadcast_to([16, 16, 8]))
        for hr in range(H_REL):
            up = psp1.tile([16, 128], F32, tag="sm_ps16")
            cp = psp1.tile([16, 1], F32, tag="sm_psc16")
            nc.tensor.matmul(up[0:16, :], qrbT[hr][:, TT - 16 : TT], rkwT[hr])
            nc.tensor.matmul(up[0:8, :], qrbT[hr][:, 192:200], rkwT[hr])
            nc.tensor.matmul(cp[0:16, :], qrbT[hr][:, TT - 16 : TT],
                             small_cols[("rkb", hr)])
            nc.tensor.matmul(cp[0:8, :], qrbT[hr][:, 192:200],
                             small_cols[("rkb", hr)])
            t16 = keep.tile([16, 128], BF16, tag=f"u16_{hr}")
            nc.scalar.copy(t16, up)
            u16[hr] = t16
            tc16 = keep.tile([16, 1], F32, tag=f"c16_{hr}")
            nc.scalar.copy(tc16, cp)
            c16[hr] = tc16
        # replicate u16 8x along partitions: u_rep[jq*16 + r] = u16[r]
        u_rep = {}
        for hr in range(H_REL):
            urp = psp1.tile([128, 128], F32, tag="sm_ps")
            nc.tensor.matmul(urp, rep16, u16[hr])
            t = keep.tile([128, 128], BF16, tag=f"urep{hr}")
            nc.scalar.copy(t, urp)
            u_rep[hr] = t

        # ---- rest of prep (scalar ring) ----
        xkT = transpose_in(key, "xkT", dma_scalar)
        xvT = transpose_in(value, "xvT", dma_scalar)

        wk = load_const("wk", io["Wk"], [128, 128])
        wv = load_const("wv", io["Wv"], [128, 128])
        wo = load_const("wo", io["Wo"], [128, 128])
        bk_c = load_const("bk", io["bk"], [128, 1])
        bv_b = consts.tile([128, 128], F32, tag="bv_b")
        bv_ap = io["bv"]
        dma_scalar(bv_b, bass.AP(tensor=bv_ap.tensor, offset=bv_ap.offset,
                                 ap=[[0, 128]] + bv_ap.ap))
        bo_b = consts.tile([128, 128], F32, tag="bo_b")
        bo_ap = io["bo"]
        dma_scalar(bo_b, bass.AP(tensor=bo_ap.tensor, offset=bo_ap.offset,
                                 ap=[[0, 128]] + bo_ap.ap))

        abs_w = {}
        for hh in range(H_ABS):
            abs_w[("aqw", hh)] = load_const(f"aqw{hh}", io["abs_q_w"][hh], [128, DK])
            abs_w[("akw", hh)] = load_const(f"akw{hh}", io["abs_k_w"][hh], [128, DK])
            small_cols[("akb", hh)] = load_const(
                f"akb{hh}", io["abs_k_b"][hh], [DK, 1])
            t = load_const(f"aqb{hh}", io["abs_q_b"][hh], [DK, 1])
            ts_ = consts.tile([DK, 1], F32, tag=f"aqb_s{hh}")
            nc.scalar.activation(ts_, t, AF.Copy, scale=SCALE)
            small_cols[("aqb_s", hh)] = ts_

        kT = {}
        for h in range(H):
            kp = psp.tile([DK, TT], F32, tag="mm_ps")
            nc.tensor.matmul(kp, wk[:, DK * h : DK * (h + 1)], xkT)
            t = keep.tile([DK, TT], F32, tag=f"kT{h}")
            nc.scalar.activation(t, kp, AF.Identity,
                                 bias=bk_c[DK * h : DK * (h + 1)])
            kT[h] = t

        # v without bias: softmax rows sum to 1, so bv folds into the output
        # bias: y = (p@v0) @ Wo + (bv @ Wo + bo)
        vb = {}
        for b in range(BPC):
            for jb, (j0, jl) in enumerate(IBLOCKS):
                vp = psp1.tile([128, 128], F32, tag="sm_ps")
                nc.tensor.matmul(vp[:jl, :], xvT[:, b * T + j0 : b * T + j0 + jl], wv)
                t = keep.tile([128, 128], BF16, tag=f"v{b}_{jb}")
                nc.scalar.copy(t[:jl, :], vp[:jl, :])
                vb[(b, jb)] = t

        qaT = {}
        kaT = {}
        for hh in range(H_ABS):
            akT = transpose_in(absk[hh].flatten_outer_dims(), f"akT{hh}",
                               dma_scalar)
            pp = psp.tile([DK, TT], F32, tag="mm_ps")
            nc.tensor.matmul(pp, abs_w[("aqw", hh)], akT)
            pqT = prep.tile([DK, TT], F32, tag="pqT")
            nc.scalar.activation(pqT, pp, AF.Identity,
                                 bias=small_cols[("aqb_s", hh)], scale=SCALE)
            t = keep.tile([DK, TT], F32, tag=f"qaT{hh}")
            nc.vector.tensor_add(t, qT[hh], pqT)
            qaT[hh] = t

            pp2 = psp.tile([DK, TT], F32, tag="mm_ps")
            nc.tensor.matmul(pp2, abs_w[("akw", hh)], akT)
            pkT = prep.tile([DK, TT], F32, tag="pqT")
            nc.scalar.activation(pkT, pp2, AF.Identity,
                                 bias=small_cols[("akb", hh)])
            t = keep.tile([DK, TT], F32, tag=f"kaT{hh}")
            nc.vector.tensor_add(t, kT[hh], pkT)
            kaT[hh] = t

        # mask tiles: (b, ib) blocks for abs heads
        mb_abs = {}
        for b in range(BPC):
            for ib, (i0, il) in enumerate(IBLOCKS):
                mi = prep.tile([128, T], I32, tag="m_i32")
                nc.gpsimd.dma_start(mi[:il, :], mask[b, 0, i0 : i0 + il, :])
                t = keep.tile([128, T], F32, tag=f"mb{b}_{ib}")
                nc.vector.tensor_scalar(t[:il, :], mi[:il, :], 1e9, -1e9,
                                        OP.mult, OP.add)
                mb_abs[(b, ib)] = t

        # mask tiles for stream blocks: partition = b*64 + (i - i0)
        mb_blk = {}
        for i0 in FULL_I0:
            mi = prep.tile([128, T], I32, tag="ms_i32")
            for b in range(BPC):
                nc.gpsimd.dma_start(mi[b * HB : (b + 1) * HB, :],
                                    mask[b, 0, i0 : i0 + HB, :])
            t = keep.tile([128, T], F32, tag=f"mbs{i0}")
            nc.vector.tensor_scalar(t, mi, 1e9, -1e9, OP.mult, OP.add)
            mb_blk[i0] = t
        mi = prep.tile([16, T], I32, tag="ms_i32l")
        for b in range(BPC):
            nc.gpsimd.dma_start(mi[b * DN : (b + 1) * DN, :],
                                mask[b, 0, 192:T, :])
        mb16 = keep.tile([16, T], F32, tag="mb16")
        nc.vector.tensor_scalar(mb16, mi, 1e9, -1e9, OP.mult, OP.add)

    # ---------------- main phase ----------------
    stream = ctx.enter_context(tc.tile_pool(name="stream", bufs=6))
    wpool = ctx.enter_context(tc.tile_pool(name="wpool", bufs=2))
    tree = ctx.enter_context(tc.tile_pool(name="tree", bufs=2))
    s2pool = ctx.enter_context(tc.tile_pool(name="s2pool", bufs=1))
    sm = ctx.enter_context(tc.tile_pool(name="sm", bufs=2))
    ps_s1 = ctx.enter_context(tc.tile_pool(name="ps_s1", bufs=2, space="PSUM"))
    ps_tp = ctx.enter_context(tc.tile_pool(name="ps_tp", bufs=2, space="PSUM"))
    ps_x = ctx.enter_context(tc.tile_pool(name="ps_x", bufs=1, space="PSUM"))
    ps_tail = ctx.enter_context(tc.tile_pool(name="ps_tail", bufs=1, space="PSUM"))

    chunk_n = [0]

    def stream_chunk(dram_aps, s2t, ub, jslice, jcw):
        """One [128, jcw, 128] chunk: dma halves, mult by ub, d-tree-reduce.
        dram_aps: list of (partition_slice, src_ap) — each src must have a
        single leading partition dim so the DMA spreads across engines."""
        rk = stream.tile([128, JC, 128], BF16, tag="rk")
        chunk_n[0] += 1
        dma_eng = nc.sync if chunk_n[0] % 2 == 0 else nc.scalar
        for psl, src in dram_aps:
            dma_eng.dma_start(rk[psl, :jcw, :], src)
        w = wpool.tile([128, JC, 128], BF16, tag="w")
        nc.vector.tensor_tensor(
            w[:, :jcw, :], rk[:, :jcw, :],
            ub.unsqueeze(1).broadcast_to([128, jcw, 128]), op=OP.mult)
        cur = w
        width = 64
        while width >= 2:
            nxt = tree.tile([128, JC, width], BF16, tag=f"L{width}")
            nc.vector.tensor_add(nxt[:, :jcw, :], cur[:, :jcw, 0:width],
                                 cur[:, :jcw, width : 2 * width])
            cur = nxt
            width //= 2
        nc.vector.tensor_add(s2t[:, jslice], cur[:, :jcw, 0],
                             cur[:, :jcw, 1])

    # PSUM x accumulator: one bank, columns (b*2+ib)*128 + h*DK per head
    x_all = ps_x.tile([128, 512], F32, tag="x_all", name="x_all")

    def x_col(b, ib, h):
        return (b * 2 + ib) * 128 + DK * h

    def softmax_pv(st, rows, segs, h_of_seg):
        """exp + rowsum + normalize + transpose/pv for one score block.
        st: [rows, T] logits (SBUF). segs: [(off, ln, b, i0)]."""
        p = sm.tile([128, T], BF16, tag="p")
        rsum = sm.tile([128, 1], F32, tag="rsum")
        nc.scalar.activation(p[:rows, :], st[:rows, :], AF.Exp,
                             accum_out=rsum[:rows])
        rcp = sm.tile([128, 1], F32, tag="rcp")
        nc.vector.reciprocal(rcp[:rows], rsum[:rows])
        pn = sm.tile([128, T], BF16, tag="pn")
        nc.scalar.activation(pn[:rows, :], p[:rows, :], AF.Copy,
                             scale=rcp[:rows])
        for (off, ln, b, i0), h in zip(segs, h_of_seg):
            ib = 0 if i0 < 128 else 1
            xoff = i0 - ib * 128
            xc = x_col(b, ib, h)
            for jb, (j0, jl) in enumerate(IBLOCKS):
                al = (off // 64) * 64  # 64-aligned covering slice for PE read
                ln_c = off + ln - al
                tp = ps_tp.tile([128, 128], BF16, tag="tp")
                nc.tensor.transpose(tp[:jl, :ln_c],
                                    pn[al : off + ln, j0 : j0 + jl],
                                    identb[al : off + ln, al : off + ln])
                pT = sm.tile([128, 128], BF16, tag="pT")
                nc.scalar.copy(pT[:jl, :ln_c], tp[:jl, :ln_c])
                nc.tensor.matmul(x_all[xoff : xoff + ln, xc : xc + DK],
                                 pT[:jl, off - al : off - al + ln],
                                 vb[(b, jb)][:jl, DK * h : DK * (h + 1)],
                                 start=(jb == 0), stop=(jb == 1))

    # ---- abs-head scores: emitted interleaved between stream blocks so the
    # DVE queue head stays on stream work at startup ----
    abs_jobs = [(b, ib, h) for b in range(BPC) for ib in range(2)
                for h in range(H_ABS)]

    def emit_abs(n):
        for _ in range(n):
            if not abs_jobs:
                return
            b, ib, h = abs_jobs.pop(0)
            i0, il = IBLOCKS[ib]
            tsl = slice(b * T + i0, b * T + i0 + il)
            s1 = ps_s1.tile([128, T], F32, tag="s1")
            nc.tensor.matmul(s1[:il, :], qaT[h][:, tsl],
                             kaT[h][:, b * T : (b + 1) * T])
            st = sm.tile([128, T], F32, tag="st")
            nc.vector.tensor_add(st[:il, :], s1[:il, :],
                                 mb_abs[(b, ib)][:il, :])
            softmax_pv(st, il, [(0, il, b, i0)], [h])

    # ---- the stream + rel scores ----
    def rel_scores_full(hr, i0, s2t):
        h = H_ABS + hr
        s1 = ps_s1.tile([128, T], F32, tag="s1")
        for b in range(BPC):
            nc.tensor.matmul(s1[b * HB : (b + 1) * HB, :],
                             qT[h][:, b * T + i0 : b * T + i0 + HB],
                             kT[h][:, b * T : (b + 1) * T])
        st = sm.tile([128, T], F32, tag="st")
        nc.vector.scalar_tensor_tensor(st, s1, c_blk[(hr, i0)], s2t,
                                       op0=OP.add, op1=OP.add)
        nc.vector.tensor_add(st, st, mb_blk[i0])
        softmax_pv(st, 128,
                   [(b * HB, HB, b, i0) for b in range(BPC)], [h, h])

    rel_stride = T * D  # row stride in rel_kernel elements
    for hr in range(H_REL):
        base = relk[hr]  # [2, 200, 200, 128] -> b, i, j, d
        for i0 in FULL_I0:
            s2t = s2pool.tile([128, T], F32, tag=f"s2_{hr}_{i0}",
                              name=f"s2_{hr}_{i0}")
            for jc0 in range(0, T, JC):
                # partition = (b in 2, i-i0 in 64); free = (j in 50, d)
                aps = []
                for b in range(BPC):
                    src = bass.AP(
                        tensor=base.tensor,
                        offset=(base.offset + (b * T + i0) * rel_stride
                                + jc0 * D),
                        ap=[[rel_stride, HB], [D, JC], [1, D]])
                    aps.append((slice(b * HB, (b + 1) * HB), src))
                stream_chunk(aps, s2t, u_blk[(hr, i0)],
                             slice(jc0, jc0 + JC), JC)
            rel_scores_full(hr, i0, s2t)
            emit_abs(1 if (hr, i0) != (0, 0) else 2)

        # block D: partition p = b*64 + r*8 + jq; free = (j' 25, d).
        # r and jq strides merge: r-stride (T*D) = 8 * jq-stride (DJ*D).
        s2p = s2pool.tile([128, DJ], F32, tag=f"s2p_{hr}", name=f"s2p_{hr}")
        aps = []
        for b in range(BPC):
            src = bass.AP(
                tensor=base.tensor,
                offset=base.offset + (b * T + 192) * rel_stride,
                ap=[[DJ * D, DN * 8], [1, DJ * D]])
            aps.append((slice(b * HB, (b + 1) * HB), src))
        stream_chunk(aps, s2p, u_rep[hr], slice(0, DJ), DJ)
        # unpack: s2_16[row, jq*25+j'] = s2p[8*row + jq, j']
        # lhsT for jq = ident columns [jq::8][:16] (stride-8 column view)
        s2l_ps = ps_tail.tile([16, T], F32, tag="s2l")
        for jq in range(8):
            e_jq = bass.AP(tensor=ident.tensor, offset=ident.offset + jq,
                           ap=[[128, 128], [8, 16]])
            nc.tensor.matmul(s2l_ps[:, jq * DJ : (jq + 1) * DJ],
                             e_jq, s2p[:, 0:DJ], start=True, stop=True)
        s2_16 = sm.tile([16, T], F32, tag="s2_16")
        nc.scalar.copy(s2_16, s2l_ps)

        # block D scores: 16 rows, row = b*8 + (i-192)
        h = H_ABS + hr
        s1 = ps_s1.tile([128, T], F32, tag="s1")
        nc.tensor.matmul(s1[0:16, :], qT[h][:, TT - 16 : TT],
                         kT[h][:, T : 2 * T])
        nc.tensor.matmul(s1[0:8, :], qT[h][:, 192:200], kT[h][:, 0:T])
        st = sm.tile([16, T], F32, tag="std")
        nc.vector.scalar_tensor_tensor(st, s1[0:16, :], c16[hr], s2_16,
                                       op0=OP.add, op1=OP.add)
        nc.vector.tensor_add(st, st, mb16)
        softmax_pv(st, 16, [(b * DN, DN, b, 192) for b in range(BPC)],
                   [h, h])
        emit_abs(1)

    emit_abs(8)

    # ---- output: bo' = bv @ Wo + bo broadcast, then project ----
    bop = ps_tail.tile([1, 128], F32, tag="tail3")
    nc.tensor.matmul(bop, bv_b[:, 0:1], wo)
    bo_r = sm.tile([1, 128], F32, tag="bo_r")
    nc.vector.tensor_add(bo_r, bop, bo_b[0:1, :])
    ones1 = consts.tile([1, 128], F32, tag="ones1")
    nc.vector.memset(ones1, 1.0)
    bobp = ps_tail.tile([128, 128], F32, tag="tail3")
    nc.tensor.matmul(bobp, ones1, bo_r)
    bo_bb = sm.tile([128, 128], F32, tag="bo_bb")
    nc.scalar.copy(bo_bb, bobp)
    for b in range(BPC):
        for ib, (i0, il) in enumerate(IBLOCKS):
            xc = (b * 2 + ib) * 128
            x_sb = sm.tile([128, 128], F32, tag="x_sb")
            nc.scalar.copy(x_sb[:il, :], x_all[:il, xc : xc + 128])
            xT_ps = ps_tail.tile([128, 128], F32, tag="tail3")
            nc.tensor.transpose(xT_ps[:, :il], x_sb[:il, :], ident[:il, :il])
            xT_sb = sm.tile([128, 128], F32, tag="xT_sb")
            nc.scalar.copy(xT_sb[:, :il], xT_ps[:, :il])
            y_ps = ps_tail.tile([128, 128], F32, tag="tail3")
            nc.tensor.matmul(y_ps[:il, :], xT_sb[:, :il], wo)
            y_sb = sm.tile([128, 128], F32, tag="y_sb")
            nc.vector.tensor_add(y_sb[:il, :], y_ps[:il, :], bo_bb[:il, :])
            nc.sync.dma_start(out[b, i0 : i0 + il, :], y_sb[:il, :])


def build_nc():
    nc = bacc.Bacc(trn_type="TRN2")
    io = {}
    io["query"] = nc.dram_tensor("query", [BPC, T, D], F32, kind="ExternalInput").ap()
    io["key"] = nc.dram_tensor("key", [BPC, T, D], F32, kind="ExternalInput").ap()
    io["value"] = nc.dram_tensor("value", [BPC, T, D], F32, kind="ExternalInput").ap()
    io["mask"] = nc.dram_tensor("mask", [BPC, 1, T, T], I32, kind="ExternalInput").ap()
    io["rel_kernel"] = nc.dram_tensor(
        "rel_kernel", [H_REL, BPC, T, T, D], BF16, kind="ExternalInput"
    ).ap()
    io["abs_kernel"] = nc.dram_tensor(
        "abs_kernel", [H_ABS, BPC, T, D], F32, kind="ExternalInput"
    ).ap()
    for nm, shape in [
        ("Wq", [D, D]), ("bq", [D]), ("Wk", [D, D]), ("bk", [D]),
        ("Wv", [D, D]), ("bv", [D]),
        ("abs_q_w", [H_ABS, D, DK]), ("abs_q_b", [H_ABS, DK]),
        ("abs_k_w", [H_ABS, D, DK]), ("abs_k_b", [H_ABS, DK]),
        ("rel_k_w", [H_REL, D, DK]), ("rel_k_b", [H_REL, DK]),
        ("rel_bias", [1, H_REL, 1, DK]),
        ("Wo", [D, D]), ("bo", [D]),
    ]:
        io[nm] = nc.dram_tensor(nm, shape, F32, kind="ExternalInput").ap()
    io["out"] = nc.dram_tensor("out", [BPC, T, D], F32, kind="ExternalOutput").ap()

    with tile.TileContext(nc) as tc:
        with ExitStack() as ctx:
            build_kernel(ctx, tc, io)
    nc.compile()
    return nc


_NC_CACHE = None


def _get_nc():
    global _NC_CACHE
    if _NC_CACHE is None:
        _NC_CACHE = build_nc()
    return _NC_CACHE


def make_in_maps(inputs):
    """Shard full inputs into per-core input maps."""
    f32 = np.float32
    weights = {
        nm: np.ascontiguousarray(np.asarray(inputs[nm], dtype=f32))
        for nm in ["Wq", "bq", "Wk", "bk", "Wv", "bv", "abs_q_w", "abs_q_b",
                   "abs_k_w", "abs_k_b", "rel_k_w", "rel_k_b", "rel_bias",
                   "Wo", "bo"]
    }
    query = np.asarray(inputs["query"], dtype=f32)
    key = np.asarray(inputs["key"], dtype=f32)
    value = np.asarray(inputs["value"], dtype=f32)
    mask = np.asarray(inputs["mask"], dtype=np.int32)
    import ml_dtypes
    relk = np.asarray(inputs["rel_kernel"], dtype=f32).astype(ml_dtypes.bfloat16)
    absk = np.asarray(inputs["abs_kernel"], dtype=f32)

    in_maps = []
    for c in range(N_CORES):
        bs = slice(c * BPC, (c + 1) * BPC)
        m = dict(weights)
        m["query"] = np.ascontiguousarray(query[bs])
        m["key"] = np.ascontiguousarray(key[bs])
        m["value"] = np.ascontiguousarray(value[bs])
        m["mask"] = np.ascontiguousarray(mask[bs])
        m["rel_kernel"] = np.ascontiguousarray(relk[:, bs])
        m["abs_kernel"] = np.ascontiguousarray(absk[:, bs])
        in_maps.append(m)
    return in_maps


def kernel(**inputs) -> np.ndarray:
    nc = _get_nc()
    in_maps = make_in_maps(inputs)
    res = run_bass_kernel_spmd(nc, in_maps, core_ids=list(range(N_CORES)))
    return np.concatenate([r["out"] for r in res.results], axis=0)


if __name__ == "__main__":
    nc = build_nc()
    print("built ok")
